# revision 1
# baseline (speedup 1.0000x reference)
"""ANI AEV representation kernel for 8 Trainium2 NeuronCores — v2.

Design (data-parallel over atoms, per the sharding hint):
  - Atoms are partitioned into 8 contiguous shards of 6250; each core
    computes its (6250, 1008) AEV slice.
  - ELL-dense layout: the first ELL contributions of every output slot
    form a dense per-partition array (radial ELL=8 distances, angular
    ELL=2 rows of [mu, d0, d1]; mu = 0.95*cos(angle), gathered on host).
  - Device computes all transcendental term math:
      radial  t_r = 0.25*fc(d)*exp(-eta*(d-c_r)^2) via the multiplicative
              recurrence t_r = t_{r-1} * B(d) * K_r, B = exp(2*eta*dlt*d)
      angular f2_w by the same recurrence, f1_j = ((1+cos(th-z_j))/2)^zeta
              via mu/sig angle addition + Ln/Exp, cutoffs via the ACT Sin
              spline (cos(x) = sin(x+pi/2)), then the 8x4 outer product.
    All Sin evaluations are batched in one phase so the ACT table set is
    switched exactly once (sin-set -> exp/ln-set).
  - Outputs are bf16 (halves HBM write traffic); host upcasts.
  - Slots with more than ELL contributions get their overflow summed
    on-device into dense per-slot rows (bucketed by count) written to a
    side buffer; the host adds those rows into the final array during
    unsharding (~1% of slots), so no scatter DMA is needed.
"""

import os
import sys

sys.path.insert(0, "/opt/trn_rl_repo")

import numpy as np

import concourse.bass as bass
import concourse.mybir as mybir
from concourse.library_overlay import lower_extended_insts
from concourse.bass_utils import run_bass_kernel_spmd
from concourse.tile import TileContext

# ---- problem constants (must match reference.py) ----
N = 50000
NCORE = 8
NB = N // NCORE          # 6250 atoms per core
S = 7
NRBF = 16
RC = 0.51
RMIN = 0.08
RCA = 0.35
RAMIN = 0.08
NA = 8
NZ = 4
ETA_R = 1970.0
ETA_A = 1250.0
ZETA = 14.1
NPAIRS = S * (S + 1) // 2   # 28
SUB = NA * NZ               # 32

RSLOTS = NB * S             # 43750 radial slots per core
ASLOTS = NB * NPAIRS        # 175000 angular slots per core

ELL_R = 8
ELL_A = 2
M_R = 16                    # radial slots per partition per block
M_A = 96                    # angular slots per partition per block
E_R = ELL_R * M_R           # 128 entries/partition per radial block
E_A = ELL_A * M_A           # 192 entries/partition per angular block
W_VEC = 5                   # outer product: w in [0,W_VEC) vector, rest gpsimd

EBUCKETS = (1, 2, 4, 8, 16)
EMAX = EBUCKETS[-1]

F32 = mybir.dt.float32
BF16 = mybir.dt.bfloat16
AF = mybir.ActivationFunctionType
OP = mybir.AluOpType

CENTERS_R = (RMIN + (RC - RMIN) / NRBF * np.arange(NRBF)).astype(np.float64)
DLT_R = (RC - RMIN) / NRBF
SHFA = (RAMIN + (RCA - RAMIN) / NA * np.arange(NA)).astype(np.float64)
DLT_A = (RCA - RAMIN) / NA
SHFZ = ((np.arange(NZ) + 0.5) * (np.pi / NZ)).astype(np.float64)
COSZ = np.cos(SHFZ)
SINZ = np.sin(SHFZ)

# radial log-space chain: log t_r = log t_{r-1} + v + KL_r with
# v = 2*eta*dlt*(d - c0), KL_r = -eta*dlt^2*(2r-1)
B_R_SCALE = float(2.0 * ETA_R * DLT_R)
CV_R = float(-2.0 * ETA_R * DLT_R * CENTERS_R[0])
KL_R = [float(-ETA_R * DLT_R * DLT_R * (2 * r - 1)) for r in range(1, NRBF)]
# angular f2 multiplicative chain, shifted by e^SHIFT_A to stay normal:
# f2s_w = f2s_{w-1} * Ba * K_w, Ba = exp(eta*dlt*davsum)
K_A = [float(np.exp(-ETA_A * DLT_A * (SHFA[w] + SHFA[w - 1])))
       for w in range(1, NA)]
B_A_SCALE = float(ETA_A * DLT_A)   # applied to davsum = d0 + d1
SHIFT_A = 45.0                     # f2 carries e^+S, tjf carries e^-S

SIN_SCALE_R = float(np.pi / (2.0 * RC))
SIN_SCALE_A = float(np.pi / (2.0 * RCA))
HALFPI = float(np.pi / 2.0)

INERT_D_R = 0.7             # pads: B finite, exp(-eta*(d-c)^2) == 0
INERT_A = np.array([0.0, 0.9, 0.9], np.float32)   # [mu, d0, d1]


def _triu_index_np(num_species):
    s1, s2 = np.triu_indices(num_species)
    ret = np.zeros((num_species, num_species), dtype=np.int64)
    ret[s1, s2] = np.arange(len(s1))
    ret[s2, s1] = np.arange(len(s1))
    return ret


TRIU = _triu_index_np(S)

# --------------------------------------------------------------------------
# Host planning
# --------------------------------------------------------------------------


def _blocks(total, m):
    out = []
    off = 0
    while off < total:
        rem = total - off
        if rem >= 128 * m:
            out.append((off, 128, m))
            off += 128 * m
        else:
            p = rem // m
            if p > 0:
                out.append((off, p, m))
                off += p * m
            if total - off > 0:
                out.append((off, 1, total - off))
                off = total
    return out


BLOCKS_R = _blocks(RSLOTS, M_R)
BLOCKS_A = _blocks(ASLOTS, M_A)


def _block_meta(blocks, ell):
    meta = []
    e0 = 0
    for (off, P_, M_) in blocks:
        E = ell * M_
        meta.append((off, P_, M_, E, e0))
        e0 += E
    return meta


META_R = _block_meta(BLOCKS_R, ELL_R)
META_A = _block_meta(BLOCKS_A, ELL_A)
PPT_R = sum(m[3] for m in META_R)
PPT_A = sum(m[3] for m in META_A)


def _plan_dense(slots, vals, nslots, ell, blocks, inert_row):
    """First `ell` contributions per slot -> block-aware dense array;
    the rest become extras."""
    order = np.argsort(slots, kind="stable")
    ss = slots[order]
    vs = vals[order]
    counts = np.bincount(ss, minlength=nslots)
    starts = np.zeros(nslots + 1, np.int64)
    np.cumsum(counts, out=starts[1:])
    rank = np.arange(len(ss)) - np.repeat(starts[:-1], counts)

    dense = np.tile(inert_row.astype(np.float32), (nslots * ell, 1))
    keep = rank < ell
    offs = np.array([b[0] for b in blocks])
    bidx = np.searchsorted(offs, ss[keep], side="right") - 1
    boff = offs[bidx]
    bM = np.array([b[2] for b in blocks])[bidx]
    p = (ss[keep] - boff) // bM
    mm = (ss[keep] - boff) % bM
    pos = boff * ell + p * (ell * bM) + rank[keep] * bM + mm
    dense[pos] = vs[keep]
    ex = ~keep
    return dense, (ss[ex], (rank[ex] - ell).astype(np.int64), vs[ex])


def _pack_ppt(dense, meta, ell, inert_row):
    C = dense.shape[1]
    parts = []
    for (off, P_, M_, E, e0) in meta:
        a = dense[off * ell:(off + P_ * M_) * ell].reshape(P_, E, C)
        if P_ < 128:
            pad = np.tile(inert_row.astype(np.float32), (128 - P_, E, 1))
            a = np.concatenate([a, pad], axis=0)
        parts.append(a)
    return np.concatenate(parts, axis=1)    # (128, PPT, C)


def _plan_rows(ex_slot, ex_rank, ex_vals, inert_row):
    """Overflow contributions -> rows of <=EMAX entries bucketed by count.
    Returns {e: (rows (n,e,C) f32, row_slot (n,))}, rows sorted by slot."""
    out = {}
    if len(ex_slot) == 0:
        return out
    row_id = ex_rank // EMAX
    within = ex_rank % EMAX
    key = ex_slot * 64 + row_id
    ukey, uinv = np.unique(key, return_inverse=True)
    u_slot = ukey // 64
    row_n = np.bincount(uinv)
    barr = np.asarray(EBUCKETS)
    row_e = barr[np.searchsorted(barr, row_n)]
    for e in EBUCKETS:
        rows_mask = row_e == e
        nrows = int(rows_mask.sum())
        if nrows == 0:
            continue
        ridx = np.nonzero(rows_mask)[0]
        rmap = np.full(len(ukey), -1, np.int64)
        rmap[ridx] = np.arange(nrows)
        cmask = rmap[uinv] >= 0
        rows = np.tile(inert_row.astype(np.float32), (nrows, e, 1))
        rows[rmap[uinv[cmask]], within[cmask]] = ex_vals[cmask]
        out[e] = (rows, u_slot[ridx])
    return out


def _chunk_table(per_core_rows, chk_entries):
    """Global chunk list [(e, n_pad)] with n_pad rows <= 128*(chk//e),
    plus per-core per-chunk (rows, slots)."""
    table = []
    core_chunks = [[] for _ in per_core_rows]
    for e in EBUCKETS:
        nmax = max((len(rc[e][1]) if e in rc else 0) for rc in per_core_rows)
        if nmax == 0:
            continue
        n_pad_total = ((nmax + 127) // 128) * 128
        cap = 128 * (chk_entries // e)
        start = 0
        while start < n_pad_total:
            n_pad = min(cap, n_pad_total - start)
            table.append((e, n_pad))
            for ci, rc in enumerate(per_core_rows):
                rows, slots = rc.get(e, (np.zeros((0, e, 0), np.float32),
                                         np.zeros(0, np.int64)))
                core_chunks[ci].append((rows[start:start + n_pad],
                                        slots[start:start + n_pad]))
            start += n_pad
    return table, core_chunks


def _sec_device_layout(rows, n_pad, e, inert_row):
    """(n, e, C) rows -> flat (128*C*rpp*e): row q -> partition q%128,
    row-slot q//128; per-partition comp-major."""
    C = len(inert_row)
    n = rows.shape[0]
    full = np.tile(inert_row.astype(np.float32), (n_pad, e, 1))
    if n:
        full[:n] = rows
    rpp = n_pad // 128
    arr = full.reshape(rpp, 128, e, C).transpose(1, 3, 0, 2)
    return np.ascontiguousarray(arr).reshape(-1)


def _prepare(inputs):
    atom_index = np.asarray(inputs["atom_index"])
    pair_indices = np.asarray(inputs["pair_indices"])
    d_ij = np.asarray(inputs["d_ij"])
    r_ij = np.asarray(inputs["r_ij"])
    central = np.asarray(inputs["central_atom_index"])
    p12 = np.asarray(inputs["pair_index12"])
    sign12 = np.asarray(inputs["sign12"])

    i, j = pair_indices[0], pair_indices[1]
    si, sj = atom_index[i], atom_index[j]
    d = d_ij[:, 0].astype(np.float32)

    dest = np.concatenate([i, j])
    osp = np.concatenate([sj, si]).astype(np.int64)
    dval = np.concatenate([d, d]).astype(np.float32)[:, None]

    p0, p1 = p12[0], p12[1]
    v0 = r_ij[p0] * sign12[0].astype(np.float32)[:, None]
    v1 = r_ij[p1] * sign12[1].astype(np.float32)[:, None]
    d0 = d[p0]
    d1 = d[p1]
    mu = 0.95 * np.einsum("ij,ij->i", v0, v1) / (d0 * d1)
    s0 = np.where(sign12[0] == 1, sj[p0], si[p0])
    s1 = np.where(sign12[1] == 1, sj[p1], si[p1])
    cls = TRIU[s0, s1].astype(np.int64)
    geom = np.stack([mu, d0, d1], axis=1).astype(np.float32)

    inert_r = np.array([INERT_D_R], np.float32)
    dense_cores = []
    rows_r_cores = []
    rows_a_cores = []
    for c in range(NCORE):
        base = c * NB
        m = (dest >= base) & (dest < base + NB)
        slot_r = ((dest[m] - base) * S + osp[m]).astype(np.int64)
        dense_r, ex_r = _plan_dense(slot_r, dval[m], RSLOTS, ELL_R,
                                    BLOCKS_R, inert_r)
        rows_r_cores.append(_plan_rows(*ex_r, inert_r))

        m = (central >= base) & (central < base + NB)
        slot_a = ((central[m] - base) * NPAIRS + cls[m]).astype(np.int64)
        dense_a, ex_a = _plan_dense(slot_a, geom[m], ASLOTS, ELL_A,
                                    BLOCKS_A, INERT_A)
        rows_a_cores.append(_plan_rows(*ex_a, INERT_A))
        dense_cores.append((dense_r, dense_a))

    table_r, chunks_r = _chunk_table(rows_r_cores, E_R)
    table_a, chunks_a = _chunk_table(rows_a_cores, E_A)

    in_maps = []
    merge = []
    for c in range(NCORE):
        dense_r, dense_a = dense_cores[c]
        mu_ppt = _pack_ppt(dense_a[:, 0:1], META_A, ELL_A, INERT_A[0:1])
        d01_parts = []
        for (off, P_, M_, E, e0) in META_A:
            a = dense_a[off * ELL_A:(off + P_ * M_) * ELL_A, 1:3] \
                .reshape(P_, E, 2)
            if P_ < 128:
                pad = np.tile(INERT_A[1:3], (128 - P_, E, 1))
                a = np.concatenate([a, pad], axis=0)
            d01_parts.append(np.ascontiguousarray(
                a.transpose(0, 2, 1)).reshape(-1))   # [p][comp][E] per block
        d01 = np.concatenate(d01_parts)
        dr_parts = []
        for (off, P_, M_, E, e0) in META_R:
            a = dense_r[off * ELL_R:(off + P_ * M_) * ELL_R].reshape(P_, E)
            if P_ < 128:
                a = np.concatenate(
                    [a, np.full((128 - P_, E), INERT_D_R, np.float32)],
                    axis=0)
            dr_parts.append(np.ascontiguousarray(a).reshape(-1))
        dr = np.concatenate(dr_parts)    # block-major [blk][p][E]

        gext_parts = []
        mrg_a = []
        for ti, (e, n_pad) in enumerate(table_a):
            rows, rslot = chunks_a[c][ti]
            gext_parts.append(_sec_device_layout(rows, n_pad, e, INERT_A))
            mrg_a.append(rslot)
        drext_parts = []
        mrg_r = []
        for ti, (e, n_pad) in enumerate(table_r):
            rows, rslot = chunks_r[c][ti]
            drext_parts.append(_sec_device_layout(rows, n_pad, e, inert_r))
            mrg_r.append(rslot)

        in_maps.append({
            "mu": np.ascontiguousarray(mu_ppt[:, :, 0]).reshape(-1),
            "d01": d01,
            "dr": dr,
            "gext": (np.concatenate(gext_parts) if gext_parts
                     else np.zeros(128, np.float32)),
            "drext": (np.concatenate(drext_parts) if drext_parts
                      else np.zeros(128, np.float32)),
        })
        merge.append((mrg_r, mrg_a))

    layout = dict(
        table_r=table_r, table_a=table_a,
        gext_len=max(1, sum(3 * (n // 128) * e for (e, n) in table_a)),
        drext_len=max(1, sum((n // 128) * e for (e, n) in table_r)),
        ext_r_len=max(1, sum((n // 128) * NRBF for (e, n) in table_r)),
        ext_a_len=max(1, sum((n // 128) * SUB for (e, n) in table_a)),
    )
    return in_maps, layout, merge


# --------------------------------------------------------------------------
# Device kernel builder
# --------------------------------------------------------------------------


def build_nc(layout):
    no_gps = os.environ.get("ANI_NO_GPS") == "1"
    nc = bass.Bass()
    mu_p = nc.declare_dram_parameter("mu", [128 * PPT_A], F32, isOutput=False)
    d01_p = nc.declare_dram_parameter("d01", [128 * 2 * PPT_A], F32,
                                      isOutput=False)
    dr_p = nc.declare_dram_parameter("dr", [128 * PPT_R], F32, isOutput=False)
    gext_p = nc.declare_dram_parameter("gext", [128 * layout["gext_len"]],
                                       F32, isOutput=False)
    drext_p = nc.declare_dram_parameter("drext", [128 * layout["drext_len"]],
                                        F32, isOutput=False)
    out_r = nc.declare_dram_parameter("out_r", [RSLOTS * NRBF], BF16,
                                      isOutput=True)
    out_a = nc.declare_dram_parameter("out_a", [ASLOTS * SUB], BF16,
                                      isOutput=True)
    ext_r = nc.declare_dram_parameter("ext_r", [128 * layout["ext_r_len"]],
                                      BF16, isOutput=True)
    ext_a = nc.declare_dram_parameter("ext_a", [128 * layout["ext_a_len"]],
                                      BF16, isOutput=True)

    bias_vals = [HALFPI, 1.0, -float(CENTERS_R[0]), -float(SHFA[0]),
                 SHIFT_A, -SHIFT_A]
    for k, v in enumerate(sorted(set(bias_vals))):
        t = nc.alloc_sbuf_tensor(f"bconst{k}", [128, 1], F32)
        nc.gpsimd.memset(t.ap(), v)
        nc.const_aps.aps[(F32, v)] = t.ap()
    nc.all_engine_barrier()

    act = nc.scalar.activation
    vec = nc.vector
    gps = nc.vector if no_gps else nc.gpsimd

    with TileContext(nc) as tc:
        with tc.tile_pool(name="main", bufs=1) as pool:
            # ---------------- static loads ----------------
            mu_t = pool.tile([128, PPT_A], F32, tag="mu", name="mu")
            nc.sync.dma_start(out=mu_t[:, :],
                              in_=mu_p[:].rearrange("(p q) -> p q", p=128))
            ga_secs = []
            off = 0
            for ti, (e, n_pad) in enumerate(layout["table_a"]):
                rpp = n_pad // 128
                t = pool.tile([128, 3, rpp, e], F32, tag=f"gae{ti}",
                              name=f"gae{ti}")
                nc.sync.dma_start(
                    out=t[:, :, :, :].rearrange("p a b c -> p (a b c)"),
                    in_=gext_p[off:off + 128 * 3 * rpp * e]
                    .rearrange("(p q) -> p q", p=128))
                ga_secs.append(t)
                off += 128 * 3 * rpp * e
            dr_secs = []
            off = 0
            for ti, (e, n_pad) in enumerate(layout["table_r"]):
                rpp = n_pad // 128
                t = pool.tile([128, rpp * e], F32, tag=f"dre{ti}",
                              name=f"dre{ti}")
                nc.sync.dma_start(
                    out=t[:, :],
                    in_=drext_p[off:off + 128 * rpp * e]
                    .rearrange("(p q) -> p q", p=128))
                dr_secs.append(t)
                off += 128 * rpp * e

            # ---------------- static derived ----------------
            prod_t = pool.tile([128, PPT_A], BF16, tag="prod", name="prod")
            davs_t = pool.tile([128, PPT_A], BF16, tag="davs", name="davs")
            sig_t = pool.tile([128, PPT_A], BF16, tag="sig", name="sig")
            fc2_t = pool.tile([128, PPT_A], BF16, tag="fc2", name="fc2")
            ba_t = pool.tile([128, PPT_A], BF16, tag="ba", name="ba")
            f20_t = pool.tile([128, PPT_A], BF16, tag="f20", name="f20")
            fcs_t = pool.tile([128, PPT_R], BF16, tag="fcs", name="fcs")

            # ============ PHASE A: all Sin-table work ============
            for (off_, P_, M_, E, e0) in META_A:
                d01b = pool.tile([128, 2 * E_A], F32, tag="d01", name="d01",
                                 bufs=3)
                base = 128 * 2 * e0
                nc.sync.dma_start(
                    out=d01b[:, 0:2 * E],
                    in_=d01_p[base:base + 128 * 2 * E]
                    .rearrange("(p q) -> p q", p=128))
                s0 = pool.tile([128, E_A], BF16, tag="s0", name="s0", bufs=2)
                s1 = pool.tile([128, E_A], BF16, tag="s1", name="s1", bufs=2)
                act(s0[:, 0:E], d01b[:, 0:E], AF.Sin, scale=SIN_SCALE_A,
                    bias=HALFPI)
                act(s1[:, 0:E], d01b[:, E:2 * E], AF.Sin, scale=SIN_SCALE_A,
                    bias=HALFPI)
                vec.tensor_tensor(prod_t[:, e0:e0 + E], s0[:, 0:E],
                                  s1[:, 0:E], OP.mult)
                vec.tensor_tensor(davs_t[:, e0:e0 + E], d01b[:, 0:E],
                                  d01b[:, E:2 * E], OP.add)
            for (off_, P_, M_, E, e0) in META_R:
                drb = pool.tile([128, E_R], F32, tag="drA", name="drA",
                                bufs=3)
                nc.sync.dma_start(
                    out=drb[:, 0:E],
                    in_=dr_p[128 * e0:128 * (e0 + E)]
                    .rearrange("(p q) -> p q", p=128))
                act(fcs_t[:, e0:e0 + E], drb[:, 0:E], AF.Sin,
                    scale=SIN_SCALE_R, bias=HALFPI)
            ga_A = []
            for ti, (e, n_pad) in enumerate(layout["table_a"]):
                rpp = n_pad // 128
                Ein = rpp * e
                ga = ga_secs[ti]
                s0 = pool.tile([128, E_A], BF16, tag="s0", name="s0e",
                               bufs=2)
                s1 = pool.tile([128, E_A], BF16, tag="s1", name="s1e",
                               bufs=2)
                d0v = ga[:, 1, :, :].rearrange("p a b -> p (a b)")
                d1v = ga[:, 2, :, :].rearrange("p a b -> p (a b)")
                act(s0[:, 0:Ein], d0v, AF.Sin, scale=SIN_SCALE_A,
                    bias=HALFPI)
                act(s1[:, 0:Ein], d1v, AF.Sin, scale=SIN_SCALE_A,
                    bias=HALFPI)
                prode = pool.tile([128, Ein], BF16, tag=f"prE{ti}",
                                  name=f"prE{ti}")
                davse = pool.tile([128, Ein], BF16, tag=f"dvE{ti}",
                                  name=f"dvE{ti}")
                vec.tensor_tensor(prode[:, :], s0[:, 0:Ein], s1[:, 0:Ein],
                                  OP.mult)
                vec.tensor_tensor(davse[:, :], d0v, d1v, OP.add)
                ga_A.append((prode, davse))
            fcs_secs = []
            for ti, (e, n_pad) in enumerate(layout["table_r"]):
                rpp = n_pad // 128
                t = pool.tile([128, rpp * e], BF16, tag=f"fcE{ti}",
                              name=f"fcE{ti}")
                act(t[:, :], dr_secs[ti][:, :], AF.Sin, scale=SIN_SCALE_R,
                    bias=HALFPI)
                fcs_secs.append(t)

            # ============ batched B-prep (exp/ln table) ============
            def b_prep(mu_ap, prod_ap, davs_ap, sig_ap, fc2_ap, ba_ap,
                       f20_ap, n, tmptag, nametag):
                tmp1 = pool.tile([128, n], F32, tag=tmptag,
                                 name=nametag + "t1", bufs=2)
                tmp2 = pool.tile([128, n], F32, tag=tmptag,
                                 name=nametag + "t2", bufs=2)
                act(tmp1[:, :], mu_ap, AF.Square)
                act(tmp2[:, :], tmp1[:, :], AF.Ln, scale=-1.0, bias=1.0)
                act(sig_ap, tmp2[:, :], AF.Exp, scale=0.5)
                act(fc2_ap, prod_ap, AF.Square, scale=float(np.sqrt(2.0)))
                act(ba_ap, davs_ap, AF.Exp, scale=B_A_SCALE)
                tmp3 = pool.tile([128, n], F32, tag=tmptag,
                                 name=nametag + "t3", bufs=2)
                act(tmp3[:, :], davs_ap, AF.Square, scale=0.5,
                    bias=-float(SHFA[0]))
                # f2_0 * e^SHIFT_A (the shift is cancelled inside tjf)
                act(f20_ap, tmp3[:, :], AF.Exp, scale=-ETA_A, bias=SHIFT_A)

            HPPT = (PPT_A + 1) // 2
            for ci, (a, b) in enumerate([(0, HPPT), (HPPT, PPT_A)]):
                sl = slice(a, b)
                b_prep(mu_t[:, sl], prod_t[:, sl], davs_t[:, sl],
                       sig_t[:, sl], fc2_t[:, sl], ba_t[:, sl],
                       f20_t[:, sl], b - a, "btmp", f"bp{ci}")
            ext_B = []
            for ti, (e, n_pad) in enumerate(layout["table_a"]):
                rpp = n_pad // 128
                Ein = rpp * e
                prode, davse = ga_A[ti]
                sige = pool.tile([128, Ein], BF16, tag=f"sgE{ti}",
                                 name=f"sgE{ti}")
                fc2e = pool.tile([128, Ein], BF16, tag=f"fcE2{ti}",
                                 name=f"fcE2{ti}")
                bae = pool.tile([128, Ein], BF16, tag=f"baE{ti}",
                                name=f"baE{ti}")
                f20e = pool.tile([128, Ein], BF16, tag=f"f20E{ti}",
                                 name=f"f20E{ti}")
                mue = ga_secs[ti][:, 0, :, :].rearrange("p a b -> p (a b)")
                b_prep(mue, prode[:, :], davse[:, :], sige[:, :],
                       fc2e[:, :], bae[:, :], f20e[:, :], Ein, "betmp",
                       f"bpe{ti}")
                ext_B.append((sige, fc2e, bae, f20e))

            # ============ angular term pipeline ============
            def angular_terms(mu_ap, sig_ap, fc2_ap, ba_ap, f20_ap, E,
                              terms2):
                """APs are (128, E) views; fills terms2[:, 0:E, :]."""
                tj = pool.tile([128, NZ, E_A], F32, tag="tj", name="tj",
                               bufs=2)
                for jj in range(NZ):
                    vec.tensor_scalar(tj[:, jj, 0:E], sig_ap,
                                      0.5 * float(SINZ[jj]), 0.5,
                                      OP.mult, OP.add)
                    vec.scalar_tensor_tensor(tj[:, jj, 0:E], mu_ap,
                                             0.5 * float(COSZ[jj]),
                                             tj[:, jj, 0:E], OP.mult,
                                             OP.add)
                vec.tensor_scalar(tj[:, :, 0:E], tj[:, :, 0:E], 1e-20,
                                  None, OP.max, OP.bypass)
                act(tj[:, :, 0:E], tj[:, :, 0:E], AF.Ln)
                tjf = pool.tile([128, NZ, E_A], BF16, tag="tjf", name="tjf",
                                bufs=2)
                act(tjf[:, :, 0:E], tj[:, :, 0:E], AF.Exp, scale=ZETA,
                    bias=-SHIFT_A)
                vec.tensor_tensor(
                    tjf[:, :, 0:E], tjf[:, :, 0:E],
                    fc2_ap.unsqueeze(1).broadcast_to([128, NZ, E]), OP.mult)

                f2 = pool.tile([128, NA, E_A], BF16, tag="f2", name="f2",
                               bufs=2)
                vec.tensor_copy(f2[:, 0, 0:E], f20_ap)
                for w in range(1, NA):
                    vec.scalar_tensor_tensor(f2[:, w, 0:E], ba_ap,
                                             K_A[w - 1], f2[:, w - 1, 0:E],
                                             OP.mult, OP.mult)
                o = terms2.rearrange("p e (w j) -> p e w j", w=NA)
                f2v = f2[:, :, 0:E].transpose([0, 2, 1]).unsqueeze(3) \
                    .broadcast_to([128, E, NA, NZ])
                tjv = tjf[:, :, 0:E].transpose([0, 2, 1]).unsqueeze(2) \
                    .broadcast_to([128, E, NA, NZ])
                vec.tensor_tensor(o[:, :, 0:W_VEC, :],
                                  f2v[:, :, 0:W_VEC, :],
                                  tjv[:, :, 0:W_VEC, :], OP.mult)
                gps.tensor_tensor(o[:, :, W_VEC:NA, :],
                                  f2v[:, :, W_VEC:NA, :],
                                  tjv[:, :, W_VEC:NA, :], OP.mult)

            for (off_, P_, M_, E, e0) in META_A:
                terms2 = pool.tile([128, E_A, SUB], BF16, tag="t2",
                                   name="t2", bufs=2)
                sl = slice(e0, e0 + E)
                angular_terms(mu_t[:, sl], sig_t[:, sl], fc2_t[:, sl],
                              ba_t[:, sl], f20_t[:, sl], E,
                              terms2[:, 0:E, :])
                fin = pool.tile([128, M_A * SUB], BF16, tag="fin",
                                name="fin", bufs=2)
                finv = fin[:, :].rearrange("p (a b) -> p a b", b=SUB)
                gps.tensor_tensor(finv[:, 0:M_, :], terms2[:, 0:M_, :],
                                  terms2[:, M_:E, :], OP.add)
                dst = out_a[off_ * SUB:(off_ + P_ * M_) * SUB] \
                    .rearrange("(p q) -> p q", p=P_)
                nc.sync.dma_start(out=dst, in_=fin[:P_, 0:M_ * SUB])

            # ============ radial term pipeline (log-space chain) ========
            def radial_terms(d_ap, fcs_ap, E, logt, terms):
                sq = pool.tile([128, E_R], F32, tag="rt0", name="rt0",
                               bufs=2)
                act(sq[:, 0:E], d_ap, AF.Square, bias=-float(CENTERS_R[0]))
                lnfc = pool.tile([128, E_R], F32, tag="rfc", name="rfc",
                                 bufs=2)
                act(lnfc[:, 0:E], fcs_ap, AF.Square, scale=0.5)
                act(lnfc[:, 0:E], lnfc[:, 0:E], AF.Ln)
                vv = pool.tile([128, E_R], F32, tag="rbb", name="rbb",
                               bufs=2)
                vec.tensor_scalar(vv[:, 0:E], d_ap, B_R_SCALE, CV_R,
                                  OP.mult, OP.add)
                vec.scalar_tensor_tensor(logt[:, 0:E, 0], sq[:, 0:E],
                                         -ETA_R, lnfc[:, 0:E], OP.mult,
                                         OP.add)
                for r in range(1, NRBF):
                    vec.scalar_tensor_tensor(logt[:, 0:E, r], vv[:, 0:E],
                                             KL_R[r - 1],
                                             logt[:, 0:E, r - 1],
                                             OP.add, OP.add)
                act(terms[:, 0:E, :], logt[:, 0:E, :], AF.Exp)

            for (off_, P_, M_, E, e0) in META_R:
                drb = pool.tile([128, E_R], F32, tag="drB", name="drB",
                                bufs=3)
                nc.sync.dma_start(
                    out=drb[:, 0:E],
                    in_=dr_p[128 * e0:128 * (e0 + E)]
                    .rearrange("(p q) -> p q", p=128))
                logt = pool.tile([128, E_R, NRBF], F32, tag="rlog",
                                 name="rlog", bufs=2)
                terms = pool.tile([128, E_R, NRBF], BF16, tag="rterms",
                                  name="rterms", bufs=2)
                radial_terms(drb[:, 0:E], fcs_t[:, e0:e0 + E], E,
                             logt, terms)
                M4 = E // 2
                t4 = pool.tile([128, E_R // 2, NRBF], BF16, tag="rt4",
                               name="rt4", bufs=2)
                vec.tensor_tensor(t4[:, 0:M4, :], terms[:, 0:M4, :],
                                  terms[:, M4:E, :], OP.add)
                t2t = pool.tile([128, E_R // 4, NRBF], BF16, tag="rt2",
                                name="rt2", bufs=2)
                vec.tensor_tensor(t2t[:, 0:M4 // 2, :], t4[:, 0:M4 // 2, :],
                                  t4[:, M4 // 2:M4, :], OP.add)
                fin16 = pool.tile([128, M_R * NRBF], BF16, tag="rfin",
                                  name="rfin", bufs=2)
                f16v = fin16[:, :].rearrange("p (a b) -> p a b", b=NRBF)
                vec.tensor_tensor(f16v[:, 0:M_, :], t2t[:, 0:M_, :],
                                  t2t[:, M_:2 * M_, :], OP.add)
                dst = out_r[off_ * NRBF:(off_ + P_ * M_) * NRBF] \
                    .rearrange("(p q) -> p q", p=P_)
                nc.sync.dma_start(out=dst, in_=fin16[:P_, 0:M_ * NRBF])

            # ============ extras: angular chunks ============
            eoff = 0
            for ti, (e, n_pad) in enumerate(layout["table_a"]):
                rpp = n_pad // 128
                Ein = rpp * e
                sige, fc2e, bae, f20e = ext_B[ti]
                mue = ga_secs[ti][:, 0, :, :].rearrange("p a b -> p (a b)")
                terms2 = pool.tile([128, E_A, SUB], BF16, tag="t2",
                                   name=f"t2E{ti}", bufs=2)
                angular_terms(mue, sige[:, :], fc2e[:, :], bae[:, :],
                              f20e[:, :], Ein, terms2[:, 0:Ein, :])
                tv = terms2[:, 0:Ein, :].rearrange("p (a b) c -> p a b c",
                                                   b=e)
                ee = e
                while ee > 2:
                    vec.tensor_tensor(tv[:, :, 0:ee // 2, :],
                                      tv[:, :, 0:ee // 2, :],
                                      tv[:, :, ee // 2:ee, :], OP.add)
                    ee //= 2
                if ee == 2:
                    sums = pool.tile([128, M_A * SUB], BF16, tag="fin",
                                     name=f"smE{ti}", bufs=2)
                    sv = sums[:, :].rearrange("p (a b) -> p a b", b=SUB)
                    vec.tensor_tensor(sv[:, 0:rpp, :], tv[:, :, 0, :],
                                      tv[:, :, 1, :], OP.add)
                    src = sums[:, 0:rpp * SUB]
                else:
                    src = terms2[:, :, :].rearrange(
                        "p a b -> p (a b)")[:, 0:rpp * SUB]
                nc.sync.dma_start(
                    out=ext_a[128 * eoff:128 * (eoff + rpp * SUB)]
                    .rearrange("(p q) -> p q", p=128),
                    in_=src)
                eoff += rpp * SUB

            # ============ extras: radial chunks ============
            eoff = 0
            for ti, (e, n_pad) in enumerate(layout["table_r"]):
                rpp = n_pad // 128
                Ein = rpp * e
                logt = pool.tile([128, E_R, NRBF], F32, tag="rlog",
                                 name=f"lRE{ti}", bufs=2)
                terms = pool.tile([128, E_R, NRBF], BF16, tag="rterms",
                                  name=f"tRE{ti}", bufs=2)
                radial_terms(dr_secs[ti][:, :], fcs_secs[ti][:, :], Ein,
                             logt, terms)
                tv = terms[:, 0:Ein, :].rearrange("p (a b) c -> p a b c",
                                                  b=e)
                ee = e
                while ee > 2:
                    vec.tensor_tensor(tv[:, :, 0:ee // 2, :],
                                      tv[:, :, 0:ee // 2, :],
                                      tv[:, :, ee // 2:ee, :], OP.add)
                    ee //= 2
                sums = pool.tile([128, E_R * NRBF], BF16, tag="rsum",
                                 name=f"smR{ti}", bufs=2)
                sv = sums[:, :].rearrange("p (a b) -> p a b", b=NRBF)
                if ee == 2:
                    vec.tensor_tensor(sv[:, 0:rpp, :], tv[:, :, 0, :],
                                      tv[:, :, 1, :], OP.add)
                else:
                    vec.tensor_copy(sv[:, 0:rpp, :], tv[:, :, 0, :])
                nc.sync.dma_start(
                    out=ext_r[128 * eoff:128 * (eoff + rpp * NRBF)]
                    .rearrange("(p q) -> p q", p=128),
                    in_=sums[:, 0:rpp * NRBF])
                eoff += rpp * NRBF

    lower_extended_insts(nc)
    _split_excess_waits(nc, 1)
    return nc


def _split_excess_waits(nc, max_waits=1):
    """This neuronxcc build rejects >1 sem-wait per instruction at codegen;
    hoist extras onto preceding event-semaphore carriers."""
    for f in nc.m.functions:
        for b in f.blocks:
            idx = 0
            while idx < len(b.instructions):
                inst = b.instructions[idx]
                si = inst.sync_info
                if si is not None and len(si.on_wait) > max_waits:
                    waits = list(si.on_wait)
                    keep = waits[-max_waits:]
                    head = waits[:-max_waits]
                    at = idx
                    for i0 in range(0, len(head), max_waits):
                        chunk = head[i0:i0 + max_waits]
                        ev = mybir.InstEventSemaphore(
                            name=nc.get_next_instruction_name(), ins=[],
                            outs=[])
                        ev.engine = inst.engine
                        ev.sync_info = mybir.SyncInfo(on_wait=chunk,
                                                      on_update=[])
                        nc.register_instruction(ev)
                        b.instructions.insert(at, ev)
                        at += 1
                        idx += 1
                    si.on_wait = keep
                    inst.sync_info = si
                idx += 1


# --------------------------------------------------------------------------
# Entry point
# --------------------------------------------------------------------------

LAST_RESULT = {}


def kernel(**inputs):
    in_maps, layout, merge = _prepare(inputs)
    nc = build_nc(layout)
    trace = os.environ.get("ANI_TRACE") == "1"
    res = run_bass_kernel_spmd(nc, in_maps, core_ids=list(range(NCORE)),
                               trace=trace)
    LAST_RESULT["exec_time_ns"] = getattr(res, "exec_time_ns", None)
    LAST_RESULT["res"] = res

    parts = []
    for c in range(NCORE):
        rad = np.asarray(res.results[c]["out_r"]).astype(np.float32) \
            .reshape(RSLOTS, NRBF)
        ang = np.asarray(res.results[c]["out_a"]).astype(np.float32) \
            .reshape(ASLOTS, SUB)
        er = np.asarray(res.results[c]["ext_r"]).astype(np.float32)
        ea = np.asarray(res.results[c]["ext_a"]).astype(np.float32)
        mrg_r, mrg_a = merge[c]
        eoff = 0
        for ti, (e, n_pad) in enumerate(layout["table_r"]):
            rpp = n_pad // 128
            sums = er[128 * eoff:128 * (eoff + rpp * NRBF)] \
                .reshape(128, rpp, NRBF)
            slots = mrg_r[ti]
            if len(slots):
                q = np.arange(len(slots))
                np.add.at(rad, slots, sums[q % 128, q // 128])
            eoff += rpp * NRBF
        eoff = 0
        for ti, (e, n_pad) in enumerate(layout["table_a"]):
            rpp = n_pad // 128
            sums = ea[128 * eoff:128 * (eoff + rpp * SUB)] \
                .reshape(128, rpp, SUB)
            slots = mrg_a[ti]
            if len(slots):
                q = np.arange(len(slots))
                np.add.at(ang, slots, sums[q % 128, q // 128])
            eoff += rpp * SUB
        parts.append(np.concatenate([rad.reshape(NB, S * NRBF),
                                     ang.reshape(NB, NPAIRS * SUB)],
                                    axis=1))
    return np.concatenate(parts, axis=0).astype(np.float32)



# revision 11
# speedup vs baseline: 1.7935x; 1.7935x over previous
"""ANI AEV representation kernel for 8 Trainium2 NeuronCores — v3.

Design (data-parallel over atoms, per the sharding hint):
  - Atoms are partitioned into 8 contiguous shards of 6250; each core
    computes its (6250, 1008) AEV slice.
  - Angular: ELL=1 dense layout (one contribution per slot; overflow goes
    through bucketed extra rows summed on device, merged on host).
    Terms are produced in (w,j)-major layout [p, 32, e] so the 8x4 outer
    product is ONE contiguous bf16 tensor_tensor at 2x DVE mode; the host
    permutes (wj, m) -> (m, wj) while unsharding.
  - Radial: ELL=8 dense layout, 48 slots/partition/block (384 entries) so
    the 15-step log-space recurrence amortizes instruction overhead; the
    recurrence tail + reduction tree run on GpSimd to unload the DVE.
  - All Sin evaluations batched in one phase (one ACT table switch);
    exp/ln set stays loaded for the rest.
  - Outputs are bf16 (halves HBM write traffic); host upcasts.
"""

import os
import sys

sys.path.insert(0, "/opt/trn_rl_repo")

import numpy as np

import concourse.bass as bass
import concourse.mybir as mybir
from concourse.library_overlay import lower_extended_insts
from concourse.bass_utils import run_bass_kernel_spmd
from concourse.tile import TileContext

# ---- problem constants (must match reference.py) ----
N = 50000
NCORE = 8
NB = N // NCORE          # 6250 atoms per core
S = 7
NRBF = 16
RC = 0.51
RMIN = 0.08
RCA = 0.35
RAMIN = 0.08
NA = 8
NZ = 4
ETA_R = 1970.0
ETA_A = 1250.0
ZETA = 14.1
NPAIRS = S * (S + 1) // 2   # 28
SUB = NA * NZ               # 32

RSLOTS = NB * S             # 43750 radial slots per core
ASLOTS = NB * NPAIRS        # 175000 angular slots per core

ELL_R = 8
ELL_A = 1
M_R = 48                    # radial slots per partition per block
M_A = 384                   # angular slots per partition per block
E_R = ELL_R * M_R           # 384 entries/partition per radial block
E_A = ELL_A * M_A           # 384 entries/partition per angular block

CHAIN_SPLIT = int(os.environ.get("ANI_CHAIN_SPLIT", "12"))

EBUCKETS = (1, 2, 4, 8, 16)
EMAX = EBUCKETS[-1]

F32 = mybir.dt.float32
BF16 = mybir.dt.bfloat16
AF = mybir.ActivationFunctionType
OP = mybir.AluOpType

CENTERS_R = (RMIN + (RC - RMIN) / NRBF * np.arange(NRBF)).astype(np.float64)
DLT_R = (RC - RMIN) / NRBF
SHFA = (RAMIN + (RCA - RAMIN) / NA * np.arange(NA)).astype(np.float64)
DLT_A = (RCA - RAMIN) / NA
SHFZ = ((np.arange(NZ) + 0.5) * (np.pi / NZ)).astype(np.float64)
COSZ = np.cos(SHFZ)
SINZ = np.sin(SHFZ)

# radial log-space chain: log t_r = log t_{r-1} + v + KL_r with
# v = 2*eta*dlt*(d - c0), KL_r = -eta*dlt^2*(2r-1)
B_R_SCALE = float(2.0 * ETA_R * DLT_R)
CV_R = float(-2.0 * ETA_R * DLT_R * CENTERS_R[0])
KL_R = [float(-ETA_R * DLT_R * DLT_R * (2 * r - 1)) for r in range(1, NRBF)]
# angular f2 multiplicative chain, shifted by e^SHIFT_A to stay normal:
# f2s_w = f2s_{w-1} * Ba * K_w, Ba = exp(eta*dlt*davsum)
K_A = [float(np.exp(-ETA_A * DLT_A * (SHFA[w] + SHFA[w - 1])))
       for w in range(1, NA)]
B_A_SCALE = float(ETA_A * DLT_A)   # applied to davsum = d0 + d1
SHIFT_A = 45.0                     # f2 carries e^+S, tjf carries e^-S

SIN_SCALE_R = float(np.pi / (2.0 * RC))
SIN_SCALE_A = float(np.pi / (2.0 * RCA))
HALFPI = float(np.pi / 2.0)

INERT_D_R = 0.7             # pads: B finite, exp(-eta*(d-c)^2) == 0
INERT_A = np.array([0.0, 0.9, 0.9], np.float32)   # [mu, d0, d1]


def _triu_index_np(num_species):
    s1, s2 = np.triu_indices(num_species)
    ret = np.zeros((num_species, num_species), dtype=np.int64)
    ret[s1, s2] = np.arange(len(s1))
    ret[s2, s1] = np.arange(len(s1))
    return ret


TRIU = _triu_index_np(S)

# --------------------------------------------------------------------------
# Host planning
# --------------------------------------------------------------------------


def _blocks(total, m):
    """Full (128, m) blocks; the tail is a (128, ceil(rem/128)) block over
    padded slot space so every block keeps all 128 partitions busy.
    Returns (blocks, padded_total)."""
    out = []
    off = 0
    n_full = total // (128 * m)
    for _ in range(n_full):
        out.append((off, 128, m))
        off += 128 * m
    rem = total - off
    if rem > 0:
        mt = (rem + 127) // 128
        out.append((off, 128, mt))
        off += 128 * mt
    return out, off


BLOCKS_R, RSLOTS_P = _blocks(RSLOTS, M_R)
BLOCKS_A, ASLOTS_P = _blocks(ASLOTS, M_A)


def _block_meta(blocks, ell):
    meta = []
    e0 = 0
    for (off, P_, M_) in blocks:
        E = ell * M_
        meta.append((off, P_, M_, E, e0))
        e0 += E
    return meta


META_R = _block_meta(BLOCKS_R, ELL_R)
META_A = _block_meta(BLOCKS_A, ELL_A)
PPT_R = sum(m[3] for m in META_R)
PPT_A = sum(m[3] for m in META_A)


def _plan_dense(slots, vals, nslots, ell, blocks, inert_row):
    """First `ell` contributions per slot -> block-aware dense array;
    the rest become extras."""
    order = np.argsort(slots, kind="stable")
    ss = slots[order]
    vs = vals[order]
    counts = np.bincount(ss, minlength=nslots)
    starts = np.zeros(nslots + 1, np.int64)
    np.cumsum(counts, out=starts[1:])
    rank = np.arange(len(ss)) - np.repeat(starts[:-1], counts)

    dense = np.tile(inert_row.astype(np.float32), (nslots * ell, 1))
    keep = rank < ell
    offs = np.array([b[0] for b in blocks])
    bidx = np.searchsorted(offs, ss[keep], side="right") - 1
    boff = offs[bidx]
    bM = np.array([b[2] for b in blocks])[bidx]
    p = (ss[keep] - boff) // bM
    mm = (ss[keep] - boff) % bM
    pos = boff * ell + p * (ell * bM) + rank[keep] * bM + mm
    dense[pos] = vs[keep]
    ex = ~keep
    return dense, (ss[ex], (rank[ex] - ell).astype(np.int64), vs[ex])


def _plan_rows(ex_slot, ex_rank, ex_vals, inert_row):
    """Overflow contributions -> rows of <=EMAX entries bucketed by count.
    Returns {e: (rows (n,e,C) f32, row_slot (n,))}, rows sorted by slot."""
    out = {}
    if len(ex_slot) == 0:
        return out
    row_id = ex_rank // EMAX
    within = ex_rank % EMAX
    key = ex_slot * 64 + row_id
    ukey, uinv = np.unique(key, return_inverse=True)
    u_slot = ukey // 64
    row_n = np.bincount(uinv)
    barr = np.asarray(EBUCKETS)
    row_e = barr[np.searchsorted(barr, row_n)]
    for e in EBUCKETS:
        rows_mask = row_e == e
        nrows = int(rows_mask.sum())
        if nrows == 0:
            continue
        ridx = np.nonzero(rows_mask)[0]
        rmap = np.full(len(ukey), -1, np.int64)
        rmap[ridx] = np.arange(nrows)
        cmask = rmap[uinv] >= 0
        rows = np.tile(inert_row.astype(np.float32), (nrows, e, 1))
        rows[rmap[uinv[cmask]], within[cmask]] = ex_vals[cmask]
        out[e] = (rows, u_slot[ridx])
    return out


def _chunk_table(per_core_rows, chk_entries):
    """Global chunk list [(e, n_pad)] with n_pad rows <= 128*(chk//e),
    plus per-core per-chunk (rows, slots)."""
    table = []
    core_chunks = [[] for _ in per_core_rows]
    for e in EBUCKETS:
        nmax = max((len(rc[e][1]) if e in rc else 0) for rc in per_core_rows)
        if nmax == 0:
            continue
        n_pad_total = ((nmax + 127) // 128) * 128
        cap = 128 * (chk_entries // e)
        start = 0
        while start < n_pad_total:
            n_pad = min(cap, n_pad_total - start)
            table.append((e, n_pad))
            for ci, rc in enumerate(per_core_rows):
                rows, slots = rc.get(e, (np.zeros((0, e, 0), np.float32),
                                         np.zeros(0, np.int64)))
                core_chunks[ci].append((rows[start:start + n_pad],
                                        slots[start:start + n_pad]))
            start += n_pad
    return table, core_chunks


def _sec_device_layout(rows, n_pad, e, inert_row):
    """(n, e, C) rows -> flat (128*C*rpp*e): row q -> partition q%128,
    row-slot q//128; per-partition comp-major."""
    C = len(inert_row)
    n = rows.shape[0]
    full = np.tile(inert_row.astype(np.float32), (n_pad, e, 1))
    if n:
        full[:n] = rows
    rpp = n_pad // 128
    arr = full.reshape(rpp, 128, e, C).transpose(1, 3, 0, 2)
    return np.ascontiguousarray(arr).reshape(-1)


def _prepare(inputs):
    atom_index = np.asarray(inputs["atom_index"])
    pair_indices = np.asarray(inputs["pair_indices"])
    d_ij = np.asarray(inputs["d_ij"])
    r_ij = np.asarray(inputs["r_ij"])
    central = np.asarray(inputs["central_atom_index"])
    p12 = np.asarray(inputs["pair_index12"])
    sign12 = np.asarray(inputs["sign12"])

    i, j = pair_indices[0], pair_indices[1]
    si, sj = atom_index[i], atom_index[j]
    d = d_ij[:, 0].astype(np.float32)

    dest = np.concatenate([i, j])
    osp = np.concatenate([sj, si]).astype(np.int64)
    dval = np.concatenate([d, d]).astype(np.float32)[:, None]

    p0, p1 = p12[0], p12[1]
    v0 = r_ij[p0] * sign12[0].astype(np.float32)[:, None]
    v1 = r_ij[p1] * sign12[1].astype(np.float32)[:, None]
    d0 = d[p0]
    d1 = d[p1]
    mu = 0.95 * np.einsum("ij,ij->i", v0, v1) / (d0 * d1)
    s0 = np.where(sign12[0] == 1, sj[p0], si[p0])
    s1 = np.where(sign12[1] == 1, sj[p1], si[p1])
    cls = TRIU[s0, s1].astype(np.int64)
    geom = np.stack([mu, d0, d1], axis=1).astype(np.float32)

    inert_r = np.array([INERT_D_R], np.float32)
    dense_cores = []
    rows_r_cores = []
    rows_a_cores = []
    for c in range(NCORE):
        base = c * NB
        m = (dest >= base) & (dest < base + NB)
        slot_r = ((dest[m] - base) * S + osp[m]).astype(np.int64)
        dense_r, ex_r = _plan_dense(slot_r, dval[m], RSLOTS_P, ELL_R,
                                    BLOCKS_R, inert_r)
        rows_r_cores.append(_plan_rows(*ex_r, inert_r))

        m = (central >= base) & (central < base + NB)
        slot_a = ((central[m] - base) * NPAIRS + cls[m]).astype(np.int64)
        dense_a, ex_a = _plan_dense(slot_a, geom[m], ASLOTS_P, ELL_A,
                                    BLOCKS_A, INERT_A)
        rows_a_cores.append(_plan_rows(*ex_a, INERT_A))
        dense_cores.append((dense_r, dense_a))

    table_r, chunks_r = _chunk_table(rows_r_cores, E_R)
    table_a, chunks_a = _chunk_table(rows_a_cores, E_A)

    in_maps = []
    merge = []
    for c in range(NCORE):
        dense_r, dense_a = dense_cores[c]
        # mu: global p-major (128, PPT_A); d01: per block [p][comp][E]
        mu_cols = []
        d01_parts = []
        for (off, P_, M_, E, e0) in META_A:
            a = dense_a[off * ELL_A:(off + P_ * M_) * ELL_A].reshape(P_, E, 3)
            mu_cols.append(a[:, :, 0])
            d01_parts.append(np.ascontiguousarray(
                a[:, :, 1:3].transpose(0, 2, 1)).reshape(-1))
        mu_flat = np.ascontiguousarray(
            np.concatenate(mu_cols, axis=1)).reshape(-1)
        d01 = np.concatenate(d01_parts)
        dr_parts = []
        for (off, P_, M_, E, e0) in META_R:
            a = dense_r[off * ELL_R:(off + P_ * M_) * ELL_R].reshape(P_, E)
            dr_parts.append(np.ascontiguousarray(a).reshape(-1))
        dr = np.concatenate(dr_parts)    # block-major [blk][p][E]

        gext_parts = []
        mrg_a = []
        for ti, (e, n_pad) in enumerate(table_a):
            rows, rslot = chunks_a[c][ti]
            gext_parts.append(_sec_device_layout(rows, n_pad, e, INERT_A))
            mrg_a.append(rslot)
        drext_parts = []
        mrg_r = []
        for ti, (e, n_pad) in enumerate(table_r):
            rows, rslot = chunks_r[c][ti]
            drext_parts.append(_sec_device_layout(rows, n_pad, e, inert_r))
            mrg_r.append(rslot)

        in_maps.append({
            "mu": mu_flat,
            "d01": d01,
            "dr": dr,
            "gext": (np.concatenate(gext_parts) if gext_parts
                     else np.zeros(128, np.float32)),
            "drext": (np.concatenate(drext_parts) if drext_parts
                      else np.zeros(128, np.float32)),
        })
        merge.append((mrg_r, mrg_a))

    layout = dict(
        table_r=table_r, table_a=table_a,
        gext_len=max(1, sum(3 * (n // 128) * e for (e, n) in table_a)),
        drext_len=max(1, sum((n // 128) * e for (e, n) in table_r)),
        ext_r_len=max(1, sum((n // 128) * NRBF for (e, n) in table_r)),
        ext_a_len=max(1, sum((n // 128) * SUB for (e, n) in table_a)),
    )
    return in_maps, layout, merge


# --------------------------------------------------------------------------
# Device kernel builder
# --------------------------------------------------------------------------


def build_nc(layout):
    nc = bass.Bass()
    mu_p = nc.declare_dram_parameter("mu", [128 * PPT_A], F32, isOutput=False)
    d01_p = nc.declare_dram_parameter("d01", [128 * 2 * PPT_A], F32,
                                      isOutput=False)
    dr_p = nc.declare_dram_parameter("dr", [128 * PPT_R], F32, isOutput=False)
    gext_p = nc.declare_dram_parameter("gext", [128 * layout["gext_len"]],
                                       F32, isOutput=False)
    drext_p = nc.declare_dram_parameter("drext", [128 * layout["drext_len"]],
                                        F32, isOutput=False)
    out_r = nc.declare_dram_parameter("out_r", [RSLOTS_P * NRBF], BF16,
                                      isOutput=True)
    out_a = nc.declare_dram_parameter("out_a", [ASLOTS_P * SUB], BF16,
                                      isOutput=True)
    ext_r = nc.declare_dram_parameter("ext_r", [128 * layout["ext_r_len"]],
                                      BF16, isOutput=True)
    ext_a = nc.declare_dram_parameter("ext_a", [128 * layout["ext_a_len"]],
                                      BF16, isOutput=True)

    bias_vals = [HALFPI, 1.0, -float(CENTERS_R[0]), -float(SHFA[0]),
                 SHIFT_A, -SHIFT_A]
    for k, v in enumerate(sorted(set(bias_vals))):
        t = nc.alloc_sbuf_tensor(f"bconst{k}", [128, 1], F32)
        nc.gpsimd.memset(t.ap(), v)
        nc.const_aps.aps[(F32, v)] = t.ap()
    nc.all_engine_barrier()

    act = nc.scalar.activation
    vec = nc.vector
    gps = nc.gpsimd

    with TileContext(nc) as tc:
        with tc.tile_pool(name="main", bufs=1) as pool:
            # ---------------- static loads ----------------
            mu_t = pool.tile([128, PPT_A], F32, tag="mu", name="mu")
            nc.sync.dma_start(out=mu_t[:, :],
                              in_=mu_p[:].rearrange("(p q) -> p q", p=128))
            ga_secs = []
            off = 0
            for ti, (e, n_pad) in enumerate(layout["table_a"]):
                rpp = n_pad // 128
                t = pool.tile([128, 3, rpp, e], F32, tag=f"gae{ti}",
                              name=f"gae{ti}")
                nc.sync.dma_start(
                    out=t[:, :, :, :].rearrange("p a b c -> p (a b c)"),
                    in_=gext_p[off:off + 128 * 3 * rpp * e]
                    .rearrange("(p q) -> p q", p=128))
                ga_secs.append(t)
                off += 128 * 3 * rpp * e
            dr_secs = []
            off = 0
            for ti, (e, n_pad) in enumerate(layout["table_r"]):
                rpp = n_pad // 128
                t = pool.tile([128, rpp * e], F32, tag=f"dre{ti}",
                              name=f"dre{ti}")
                nc.sync.dma_start(
                    out=t[:, :],
                    in_=drext_p[off:off + 128 * rpp * e]
                    .rearrange("(p q) -> p q", p=128))
                dr_secs.append(t)
                off += 128 * rpp * e

            # ---------------- static derived ----------------
            prod_t = pool.tile([128, PPT_A], BF16, tag="prod", name="prod")
            davs_t = pool.tile([128, PPT_A], BF16, tag="davs", name="davs")
            sig_t = pool.tile([128, PPT_A], BF16, tag="sig", name="sig")
            fc2_t = pool.tile([128, PPT_A], BF16, tag="fc2", name="fc2")
            ba_t = pool.tile([128, PPT_A], BF16, tag="ba", name="ba")
            f20_t = pool.tile([128, PPT_A], BF16, tag="f20", name="f20")
            fcs_t = pool.tile([128, PPT_R], BF16, tag="fcs", name="fcs")

            # ============ PHASE A: all Sin-table work ============
            for (off_, P_, M_, E, e0) in META_A:
                d01b = pool.tile([128, 2 * E_A], F32, tag="d01", name="d01",
                                 bufs=2)
                base = 128 * 2 * e0
                nc.sync.dma_start(
                    out=d01b[:, 0:2 * E],
                    in_=d01_p[base:base + 128 * 2 * E]
                    .rearrange("(p q) -> p q", p=128))
                s0 = pool.tile([128, E_A], BF16, tag="s0", name="s0", bufs=2)
                s1 = pool.tile([128, E_A], BF16, tag="s1", name="s1", bufs=2)
                act(s0[:, 0:E], d01b[:, 0:E], AF.Sin, scale=SIN_SCALE_A,
                    bias=HALFPI)
                act(s1[:, 0:E], d01b[:, E:2 * E], AF.Sin, scale=SIN_SCALE_A,
                    bias=HALFPI)
                vec.tensor_tensor(prod_t[:, e0:e0 + E], s0[:, 0:E],
                                  s1[:, 0:E], OP.mult)
                vec.tensor_tensor(davs_t[:, e0:e0 + E], d01b[:, 0:E],
                                  d01b[:, E:2 * E], OP.add)
            for (off_, P_, M_, E, e0) in META_R:
                drb = pool.tile([128, E_R], F32, tag="drA", name="drA",
                                bufs=2)
                nc.sync.dma_start(
                    out=drb[:, 0:E],
                    in_=dr_p[128 * e0:128 * (e0 + E)]
                    .rearrange("(p q) -> p q", p=128))
                act(fcs_t[:, e0:e0 + E], drb[:, 0:E], AF.Sin,
                    scale=SIN_SCALE_R, bias=HALFPI)
            ga_A = []
            for ti, (e, n_pad) in enumerate(layout["table_a"]):
                rpp = n_pad // 128
                Ein = rpp * e
                ga = ga_secs[ti]
                s0 = pool.tile([128, E_A], BF16, tag="s0", name="s0e",
                               bufs=2)
                s1 = pool.tile([128, E_A], BF16, tag="s1", name="s1e",
                               bufs=2)
                d0v = ga[:, 1, :, :].rearrange("p a b -> p (a b)")
                d1v = ga[:, 2, :, :].rearrange("p a b -> p (a b)")
                act(s0[:, 0:Ein], d0v, AF.Sin, scale=SIN_SCALE_A,
                    bias=HALFPI)
                act(s1[:, 0:Ein], d1v, AF.Sin, scale=SIN_SCALE_A,
                    bias=HALFPI)
                prode = pool.tile([128, Ein], BF16, tag=f"prE{ti}",
                                  name=f"prE{ti}")
                davse = pool.tile([128, Ein], BF16, tag=f"dvE{ti}",
                                  name=f"dvE{ti}")
                vec.tensor_tensor(prode[:, :], s0[:, 0:Ein], s1[:, 0:Ein],
                                  OP.mult)
                vec.tensor_tensor(davse[:, :], d0v, d1v, OP.add)
                ga_A.append((prode, davse))
            fcs_secs = []
            for ti, (e, n_pad) in enumerate(layout["table_r"]):
                rpp = n_pad // 128
                t = pool.tile([128, rpp * e], BF16, tag=f"fcE{ti}",
                              name=f"fcE{ti}")
                act(t[:, :], dr_secs[ti][:, :], AF.Sin, scale=SIN_SCALE_R,
                    bias=HALFPI)
                fcs_secs.append(t)

            # ============ batched B-prep (exp/ln table) ============
            def b_prep(mu_ap, prod_ap, davs_ap, sig_ap, fc2_ap, ba_ap,
                       f20_ap, n, tmptag, nametag):
                tmp1 = pool.tile([128, n], F32, tag=tmptag,
                                 name=nametag + "t1", bufs=2)
                tmp2 = pool.tile([128, n], F32, tag=tmptag,
                                 name=nametag + "t2", bufs=2)
                act(tmp1[:, :], mu_ap, AF.Square)
                act(tmp2[:, :], tmp1[:, :], AF.Ln, scale=-1.0, bias=1.0)
                act(sig_ap, tmp2[:, :], AF.Exp, scale=0.5)
                act(fc2_ap, prod_ap, AF.Square, scale=float(np.sqrt(2.0)))
                act(ba_ap, davs_ap, AF.Exp, scale=B_A_SCALE)
                tmp3 = pool.tile([128, n], F32, tag=tmptag,
                                 name=nametag + "t3", bufs=2)
                act(tmp3[:, :], davs_ap, AF.Square, scale=0.5,
                    bias=-float(SHFA[0]))
                # f2_0 * e^SHIFT_A (the shift is cancelled inside tjf)
                act(f20_ap, tmp3[:, :], AF.Exp, scale=-ETA_A, bias=SHIFT_A)

            HPPT = (PPT_A + 1) // 2
            for ci, (a, b) in enumerate([(0, HPPT), (HPPT, PPT_A)]):
                sl = slice(a, b)
                b_prep(mu_t[:, sl], prod_t[:, sl], davs_t[:, sl],
                       sig_t[:, sl], fc2_t[:, sl], ba_t[:, sl],
                       f20_t[:, sl], b - a, "btmp", f"bp{ci}")
            ext_B = []
            for ti, (e, n_pad) in enumerate(layout["table_a"]):
                rpp = n_pad // 128
                Ein = rpp * e
                prode, davse = ga_A[ti]
                sige = pool.tile([128, Ein], BF16, tag=f"sgE{ti}",
                                 name=f"sgE{ti}")
                fc2e = pool.tile([128, Ein], BF16, tag=f"fcE2{ti}",
                                 name=f"fcE2{ti}")
                bae = pool.tile([128, Ein], BF16, tag=f"baE{ti}",
                                name=f"baE{ti}")
                f20e = pool.tile([128, Ein], BF16, tag=f"f20E{ti}",
                                 name=f"f20E{ti}")
                mue = ga_secs[ti][:, 0, :, :].rearrange("p a b -> p (a b)")
                b_prep(mue, prode[:, :], davse[:, :], sige[:, :],
                       fc2e[:, :], bae[:, :], f20e[:, :], Ein, "betmp",
                       f"bpe{ti}")
                ext_B.append((sige, fc2e, bae, f20e))

            # ============ angular term pipeline (wj-major) ============
            def angular_terms3(mu_ap, sig_ap, fc2_ap, ba_ap, f20_ap, E,
                               terms3, nm):
                """APs are (128, E) views; terms3 is a [128, SUB, E] view."""
                tj = pool.tile([128, NZ, E_A], F32, tag="tj", name="tj" + nm,
                               bufs=1)
                for jj in range(NZ):
                    vec.tensor_scalar(tj[:, jj, 0:E], sig_ap,
                                      0.5 * float(SINZ[jj]), 0.5,
                                      OP.mult, OP.add)
                    vec.scalar_tensor_tensor(tj[:, jj, 0:E], mu_ap,
                                             0.5 * float(COSZ[jj]),
                                             tj[:, jj, 0:E], OP.mult,
                                             OP.add)
                vec.tensor_scalar(tj[:, :, 0:E], tj[:, :, 0:E], 1e-20,
                                  None, OP.max, OP.bypass)
                act(tj[:, :, 0:E], tj[:, :, 0:E], AF.Ln)
                tjf = pool.tile([128, NZ, E_A], BF16, tag="tjf",
                                name="tjf" + nm, bufs=2)
                act(tjf[:, :, 0:E], tj[:, :, 0:E], AF.Exp, scale=ZETA,
                    bias=-SHIFT_A)
                vec.tensor_tensor(
                    tjf[:, :, 0:E], tjf[:, :, 0:E],
                    fc2_ap.unsqueeze(1).broadcast_to([128, NZ, E]), OP.mult)

                f2 = pool.tile([128, NA, E_A], BF16, tag="f2",
                               name="f2" + nm, bufs=1)
                vec.tensor_copy(f2[:, 0, 0:E], f20_ap)
                for w in range(1, NA):
                    vec.scalar_tensor_tensor(f2[:, w, 0:E], ba_ap,
                                             K_A[w - 1], f2[:, w - 1, 0:E],
                                             OP.mult, OP.mult)
                o4 = terms3.rearrange("p (w j) e -> p w j e", w=NA)
                f2v = f2[:, :, 0:E].unsqueeze(2) \
                    .broadcast_to([128, NA, NZ, E])
                tjv = tjf[:, :, 0:E].unsqueeze(1) \
                    .broadcast_to([128, NA, NZ, E])
                vec.tensor_tensor(o4, f2v, tjv, OP.mult)

            def angular_block(bi):
                (off_, P_, M_, E, e0) = META_A[bi]
                terms3 = pool.tile([128, SUB, E_A], BF16, tag="t24",
                                   name="t3", bufs=2)
                sl = slice(e0, e0 + E)
                angular_terms3(mu_t[:, sl], sig_t[:, sl], fc2_t[:, sl],
                               ba_t[:, sl], f20_t[:, sl], E,
                               terms3[:, :, 0:E], f"d{bi}")
                dst = out_a[off_ * SUB:(off_ + P_ * M_) * SUB] \
                    .rearrange("(p q) -> p q", p=P_)
                nc.sync.dma_start(out=dst, in_=terms3[:P_, :, 0:M_])

            # ============ radial term pipeline (log-space chain) ========
            def radial_block(bi):
                (off_, P_, M_, E, e0) = META_R[bi]
                drb = pool.tile([128, E_R], F32, tag="drB", name="drB",
                                bufs=2)
                nc.sync.dma_start(
                    out=drb[:, 0:E],
                    in_=dr_p[128 * e0:128 * (e0 + E)]
                    .rearrange("(p q) -> p q", p=128))
                fin16 = pool.tile([128, M_R * NRBF], BF16, tag="rfin",
                                  name="rfin", bufs=2)
                radial_terms(drb[:, 0:E], fcs_t[:, e0:e0 + E], E, M_,
                             fin16, f"rd{bi}")
                dst = out_r[off_ * NRBF:(off_ + P_ * M_) * NRBF] \
                    .rearrange("(p q) -> p q", p=P_)
                nc.sync.dma_start(out=dst, in_=fin16[:P_, 0:M_ * NRBF])

            def radial_terms(d_ap, fcs_ap, E, M_, fin16, nm):
                """Dense radial: E = 8*M_ entries -> fin16[:, 0:M_*16]."""
                sq = pool.tile([128, E_R], F32, tag="rt0", name="rt0" + nm,
                               bufs=2)
                act(sq[:, 0:E], d_ap, AF.Square, bias=-float(CENTERS_R[0]))
                lnfc = pool.tile([128, E_R], F32, tag="rfc", name="rfc" + nm,
                                 bufs=2)
                act(lnfc[:, 0:E], fcs_ap, AF.Square, scale=0.5)
                act(lnfc[:, 0:E], lnfc[:, 0:E], AF.Ln)
                vv = pool.tile([128, E_R], F32, tag="rbb", name="rbb" + nm,
                               bufs=2)
                vec.tensor_scalar(vv[:, 0:E], d_ap, B_R_SCALE, CV_R,
                                  OP.mult, OP.add)
                logt = pool.tile([128, E_R, NRBF], F32, tag="t24",
                                 name="rlog" + nm, bufs=2)
                vec.scalar_tensor_tensor(logt[:, 0:E, 0], sq[:, 0:E],
                                         -ETA_R, lnfc[:, 0:E], OP.mult,
                                         OP.add)
                for r in range(1, NRBF):
                    eng = vec if r < CHAIN_SPLIT else gps
                    eng.scalar_tensor_tensor(logt[:, 0:E, r], vv[:, 0:E],
                                             KL_R[r - 1],
                                             logt[:, 0:E, r - 1],
                                             OP.add, OP.add)
                terms = pool.tile([128, E_R, NRBF], BF16, tag="rterms",
                                  name="rterms" + nm, bufs=2)
                act(terms[:, 0:E, :], logt[:, 0:E, :], AF.Exp)
                M4 = E // 2
                t4 = pool.tile([128, E_R // 2, NRBF], BF16, tag="rt4",
                               name="rt4" + nm, bufs=1)
                vec.tensor_tensor(t4[:, 0:M4, :], terms[:, 0:M4, :],
                                  terms[:, M4:E, :], OP.add)
                t2t = pool.tile([128, E_R // 4, NRBF], BF16, tag="rt2",
                                name="rt2" + nm, bufs=2)
                gps.tensor_tensor(t2t[:, 0:M4 // 2, :], t4[:, 0:M4 // 2, :],
                                  t4[:, M4 // 2:M4, :], OP.add)
                f16v = fin16[:, :].rearrange("p (a b) -> p a b", b=NRBF)
                gps.tensor_tensor(f16v[:, 0:M_, :], t2t[:, 0:M_, :],
                                  t2t[:, M_:2 * M_, :], OP.add)

            # ---- merged main loop: interleave angular + radial blocks ----
            nA, nR = len(META_A), len(META_R)
            orderA = [("A", i) for i in range(nA)]
            orderR = [("R", i) for i in range(nR)]
            merged = []
            fa = fr = 0
            while fa < nA or fr < nR:
                if fa < nA and (fr >= nR or fa * nR <= fr * nA):
                    merged.append(orderA[fa])
                    fa += 1
                else:
                    merged.append(orderR[fr])
                    fr += 1
            for kind, bi in merged:
                if kind == "A":
                    angular_block(bi)
                else:
                    radial_block(bi)

            # ============ extras: angular chunks ============
            eoff = 0
            for ti, (e, n_pad) in enumerate(layout["table_a"]):
                rpp = n_pad // 128
                Ein = rpp * e
                sige, fc2e, bae, f20e = ext_B[ti]
                mue = ga_secs[ti][:, 0, :, :].rearrange("p a b -> p (a b)")
                terms3 = pool.tile([128, SUB, E_A], BF16, tag="t24",
                                   name=f"t3E{ti}", bufs=2)
                angular_terms3(mue, sige[:, :], fc2e[:, :], bae[:, :],
                               f20e[:, :], Ein, terms3[:, :, 0:Ein],
                               f"e{ti}")
                if e == 1:
                    src = terms3[:, :, 0:rpp]
                else:
                    tv = terms3[:, :, 0:Ein].rearrange(
                        "p s (a b) -> p s a b", b=e)
                    ee = e
                    while ee > 2:
                        gps.tensor_tensor(tv[:, :, :, 0:ee // 2],
                                          tv[:, :, :, 0:ee // 2],
                                          tv[:, :, :, ee // 2:ee], OP.add)
                        ee //= 2
                    asum = pool.tile([128, SUB, rpp], BF16, tag="asum",
                                     name=f"asum{ti}", bufs=2)
                    gps.tensor_tensor(asum[:, :, :], tv[:, :, :, 0],
                                      tv[:, :, :, 1], OP.add)
                    src = asum[:, :, :]
                nc.sync.dma_start(
                    out=ext_a[128 * eoff:128 * (eoff + rpp * SUB)]
                    .rearrange("(p q) -> p q", p=128),
                    in_=src)
                eoff += rpp * SUB

            # ============ extras: radial chunks ============
            eoff = 0
            for ti, (e, n_pad) in enumerate(layout["table_r"]):
                rpp = n_pad // 128
                Ein = rpp * e
                sq = pool.tile([128, E_R], F32, tag="rt0", name=f"sqE{ti}",
                               bufs=2)
                act(sq[:, 0:Ein], dr_secs[ti][:, :], AF.Square,
                    bias=-float(CENTERS_R[0]))
                lnfc = pool.tile([128, E_R], F32, tag="rfc",
                                 name=f"lfE{ti}", bufs=2)
                act(lnfc[:, 0:Ein], fcs_secs[ti][:, :], AF.Square, scale=0.5)
                act(lnfc[:, 0:Ein], lnfc[:, 0:Ein], AF.Ln)
                vv = pool.tile([128, E_R], F32, tag="rbb", name=f"vvE{ti}",
                               bufs=2)
                vec.tensor_scalar(vv[:, 0:Ein], dr_secs[ti][:, :],
                                  B_R_SCALE, CV_R, OP.mult, OP.add)
                logt = pool.tile([128, E_R, NRBF], F32, tag="t24",
                                 name=f"lRE{ti}", bufs=2)
                vec.scalar_tensor_tensor(logt[:, 0:Ein, 0], sq[:, 0:Ein],
                                         -ETA_R, lnfc[:, 0:Ein], OP.mult,
                                         OP.add)
                for r in range(1, NRBF):
                    eng = vec if r < CHAIN_SPLIT else gps
                    eng.scalar_tensor_tensor(logt[:, 0:Ein, r], vv[:, 0:Ein],
                                             KL_R[r - 1],
                                             logt[:, 0:Ein, r - 1],
                                             OP.add, OP.add)
                terms = pool.tile([128, E_R, NRBF], BF16, tag="rterms",
                                  name=f"tRE{ti}", bufs=2)
                act(terms[:, 0:Ein, :], logt[:, 0:Ein, :], AF.Exp)
                if e == 1:
                    src = terms[:, 0:rpp, :]
                else:
                    tv = terms[:, 0:Ein, :].rearrange(
                        "p (a b) c -> p a b c", b=e)
                    ee = e
                    while ee > 2:
                        gps.tensor_tensor(tv[:, :, 0:ee // 2, :],
                                          tv[:, :, 0:ee // 2, :],
                                          tv[:, :, ee // 2:ee, :], OP.add)
                        ee //= 2
                    rsum = pool.tile([128, rpp, NRBF], BF16, tag="rsum",
                                     name=f"rsum{ti}", bufs=2)
                    gps.tensor_tensor(rsum[:, :, :], tv[:, :, 0, :],
                                      tv[:, :, 1, :], OP.add)
                    src = rsum[:, :, :]
                nc.sync.dma_start(
                    out=ext_r[128 * eoff:128 * (eoff + rpp * NRBF)]
                    .rearrange("(p q) -> p q", p=128),
                    in_=src)
                eoff += rpp * NRBF

    lower_extended_insts(nc)
    _split_excess_waits(nc, 1)
    return nc


def _split_excess_waits(nc, max_waits=1):
    """This neuronxcc build rejects >1 sem-wait per instruction at codegen;
    hoist extras onto preceding event-semaphore carriers."""
    for f in nc.m.functions:
        for b in f.blocks:
            idx = 0
            while idx < len(b.instructions):
                inst = b.instructions[idx]
                si = inst.sync_info
                if si is not None and len(si.on_wait) > max_waits:
                    waits = list(si.on_wait)
                    keep = waits[-max_waits:]
                    head = waits[:-max_waits]
                    at = idx
                    for i0 in range(0, len(head), max_waits):
                        chunk = head[i0:i0 + max_waits]
                        ev = mybir.InstEventSemaphore(
                            name=nc.get_next_instruction_name(), ins=[],
                            outs=[])
                        ev.engine = inst.engine
                        ev.sync_info = mybir.SyncInfo(on_wait=chunk,
                                                      on_update=[])
                        nc.register_instruction(ev)
                        b.instructions.insert(at, ev)
                        at += 1
                        idx += 1
                    si.on_wait = keep
                    inst.sync_info = si
                idx += 1


# --------------------------------------------------------------------------
# Entry point
# --------------------------------------------------------------------------

LAST_RESULT = {}


def kernel(**inputs):
    in_maps, layout, merge = _prepare(inputs)
    nc = build_nc(layout)
    trace = os.environ.get("ANI_TRACE") == "1"
    res = run_bass_kernel_spmd(nc, in_maps, core_ids=list(range(NCORE)),
                               trace=trace)
    LAST_RESULT["exec_time_ns"] = getattr(res, "exec_time_ns", None)
    LAST_RESULT["res"] = res

    parts = []
    for c in range(NCORE):
        rad = np.asarray(res.results[c]["out_r"]).astype(np.float32) \
            .reshape(RSLOTS_P, NRBF)
        ang_raw = np.asarray(res.results[c]["out_a"]).astype(np.float32)
        ang = np.empty((ASLOTS_P, SUB), np.float32)
        for (off, P_, M_, E, e0) in META_A:
            seg = ang_raw[off * SUB:(off + P_ * M_) * SUB] \
                .reshape(P_, SUB, M_).transpose(0, 2, 1)
            ang[off:off + P_ * M_] = seg.reshape(P_ * M_, SUB)
        er = np.asarray(res.results[c]["ext_r"]).astype(np.float32)
        ea = np.asarray(res.results[c]["ext_a"]).astype(np.float32)
        mrg_r, mrg_a = merge[c]
        eoff = 0
        for ti, (e, n_pad) in enumerate(layout["table_r"]):
            rpp = n_pad // 128
            sums = er[128 * eoff:128 * (eoff + rpp * NRBF)] \
                .reshape(128, rpp, NRBF)
            slots = mrg_r[ti]
            if len(slots):
                q = np.arange(len(slots))
                np.add.at(rad, slots, sums[q % 128, q // 128])
            eoff += rpp * NRBF
        eoff = 0
        for ti, (e, n_pad) in enumerate(layout["table_a"]):
            rpp = n_pad // 128
            sums = ea[128 * eoff:128 * (eoff + rpp * SUB)] \
                .reshape(128, SUB, rpp).transpose(0, 2, 1)
            slots = mrg_a[ti]
            if len(slots):
                q = np.arange(len(slots))
                np.add.at(ang, slots, sums[q % 128, q // 128])
            eoff += rpp * SUB
        parts.append(np.concatenate(
            [rad[:RSLOTS].reshape(NB, S * NRBF),
             ang[:ASLOTS].reshape(NB, NPAIRS * SUB)], axis=1))
    return np.concatenate(parts, axis=0).astype(np.float32)


# revision 20
# speedup vs baseline: 2.0822x; 1.1610x over previous
"""ANI AEV representation kernel for 8 Trainium2 NeuronCores — v3.

Design (data-parallel over atoms, per the sharding hint):
  - Atoms are partitioned into 8 contiguous shards of 6250; each core
    computes its (6250, 1008) AEV slice.
  - Angular: ELL=1 dense layout (one contribution per slot; overflow goes
    through bucketed extra rows summed on device, merged on host).
    Terms are produced in (w,j)-major layout [p, 32, e] so the 8x4 outer
    product is ONE contiguous bf16 tensor_tensor at 2x DVE mode; the host
    permutes (wj, m) -> (m, wj) while unsharding.
  - Radial: ELL=8 dense layout, 48 slots/partition/block (384 entries) so
    the 15-step log-space recurrence amortizes instruction overhead; the
    recurrence tail + reduction tree run on GpSimd to unload the DVE.
  - All Sin evaluations batched in one phase (one ACT table switch);
    exp/ln set stays loaded for the rest.
  - Outputs are bf16 (halves HBM write traffic); host upcasts.
"""

import os
import sys

sys.path.insert(0, "/opt/trn_rl_repo")

import numpy as np

import concourse.bass as bass
import concourse.mybir as mybir
from concourse.library_overlay import lower_extended_insts
from concourse.bass_utils import run_bass_kernel_spmd
from concourse.tile import TileContext

# ---- problem constants (must match reference.py) ----
N = 50000
NCORE = 8
NB = N // NCORE          # 6250 atoms per core
S = 7
NRBF = 16
RC = 0.51
RMIN = 0.08
RCA = 0.35
RAMIN = 0.08
NA = 8
NZ = 4
ETA_R = 1970.0
ETA_A = 1250.0
ZETA = 14.1
NPAIRS = S * (S + 1) // 2   # 28
SUB = NA * NZ               # 32

RSLOTS = NB * S             # 43750 radial slots per core
ASLOTS = NB * NPAIRS        # 175000 angular slots per core

ELL_R = 8
ELL_A = 1
M_R = 48                    # radial slots per partition per block
M_A = 384                   # angular slots per partition per block
E_R = ELL_R * M_R           # 384 entries/partition per radial block
E_A = ELL_A * M_A           # 384 entries/partition per angular block

# gpsimd cannot run scalar_tensor_tensor (walrus backend crash); 16 keeps
# the whole radial recurrence on the DVE.
CHAIN_SPLIT = int(os.environ.get("ANI_CHAIN_SPLIT", "16"))

EBUCKETS = (1, 2, 4, 8, 16)
EMAX = EBUCKETS[-1]

F32 = mybir.dt.float32
BF16 = mybir.dt.bfloat16
AF = mybir.ActivationFunctionType
OP = mybir.AluOpType

CENTERS_R = (RMIN + (RC - RMIN) / NRBF * np.arange(NRBF)).astype(np.float64)
DLT_R = (RC - RMIN) / NRBF
SHFA = (RAMIN + (RCA - RAMIN) / NA * np.arange(NA)).astype(np.float64)
DLT_A = (RCA - RAMIN) / NA
SHFZ = ((np.arange(NZ) + 0.5) * (np.pi / NZ)).astype(np.float64)
COSZ = np.cos(SHFZ)
SINZ = np.sin(SHFZ)

# radial log-space chain: log t_r = log t_{r-1} + v + KL_r with
# v = 2*eta*dlt*(d - c0), KL_r = -eta*dlt^2*(2r-1)
B_R_SCALE = float(2.0 * ETA_R * DLT_R)
CV_R = float(-2.0 * ETA_R * DLT_R * CENTERS_R[0])
KL_R = [float(-ETA_R * DLT_R * DLT_R * (2 * r - 1)) for r in range(1, NRBF)]
# angular f2 multiplicative chain, shifted by e^SHIFT_A to stay normal:
# f2s_w = f2s_{w-1} * Ba * K_w, Ba = exp(eta*dlt*davsum)
K_A = [float(np.exp(-ETA_A * DLT_A * (SHFA[w] + SHFA[w - 1])))
       for w in range(1, NA)]
B_A_SCALE = float(ETA_A * DLT_A)   # applied to davsum = d0 + d1
SHIFT_A = 45.0                     # f2 carries e^+S, tjf carries e^-S

SIN_SCALE_R = float(np.pi / (2.0 * RC))
SIN_SCALE_A = float(np.pi / (2.0 * RCA))
HALFPI = float(np.pi / 2.0)

INERT_D_R = 0.7             # pads: B finite, exp(-eta*(d-c)^2) == 0
INERT_A = np.array([0.0, 0.9, 0.9], np.float32)   # [mu, d0, d1]


def _triu_index_np(num_species):
    s1, s2 = np.triu_indices(num_species)
    ret = np.zeros((num_species, num_species), dtype=np.int64)
    ret[s1, s2] = np.arange(len(s1))
    ret[s2, s1] = np.arange(len(s1))
    return ret


TRIU = _triu_index_np(S)

# --------------------------------------------------------------------------
# Host planning
# --------------------------------------------------------------------------


def _blocks(total, m):
    """Full (128, m) blocks; the tail is a (128, ceil(rem/128)) block over
    padded slot space so every block keeps all 128 partitions busy.
    Returns (blocks, padded_total)."""
    out = []
    off = 0
    n_full = total // (128 * m)
    for _ in range(n_full):
        out.append((off, 128, m))
        off += 128 * m
    rem = total - off
    if rem > 0:
        mt = (rem + 127) // 128
        out.append((off, 128, mt))
        off += 128 * mt
    return out, off


BLOCKS_R, RSLOTS_P = _blocks(RSLOTS, M_R)
BLOCKS_A, ASLOTS_P = _blocks(ASLOTS, M_A)


def _block_meta(blocks, ell):
    meta = []
    e0 = 0
    for (off, P_, M_) in blocks:
        E = ell * M_
        meta.append((off, P_, M_, E, e0))
        e0 += E
    return meta


META_R = _block_meta(BLOCKS_R, ELL_R)
META_A = _block_meta(BLOCKS_A, ELL_A)
PPT_R = sum(m[3] for m in META_R)
PPT_A = sum(m[3] for m in META_A)


def _plan_dense(slots, vals, nslots, ell, blocks, inert_row):
    """First `ell` contributions per slot -> block-aware dense array;
    the rest become extras."""
    order = np.argsort(slots, kind="stable")
    ss = slots[order]
    vs = vals[order]
    counts = np.bincount(ss, minlength=nslots)
    starts = np.zeros(nslots + 1, np.int64)
    np.cumsum(counts, out=starts[1:])
    rank = np.arange(len(ss)) - np.repeat(starts[:-1], counts)

    dense = np.tile(inert_row.astype(np.float32), (nslots * ell, 1))
    keep = rank < ell
    offs = np.array([b[0] for b in blocks])
    bidx = np.searchsorted(offs, ss[keep], side="right") - 1
    boff = offs[bidx]
    bM = np.array([b[2] for b in blocks])[bidx]
    p = (ss[keep] - boff) // bM
    mm = (ss[keep] - boff) % bM
    pos = boff * ell + p * (ell * bM) + rank[keep] * bM + mm
    dense[pos] = vs[keep]
    ex = ~keep
    return dense, (ss[ex], (rank[ex] - ell).astype(np.int64), vs[ex])


def _plan_rows(ex_slot, ex_rank, ex_vals, inert_row):
    """Overflow contributions -> rows of <=EMAX entries bucketed by count.
    Returns {e: (rows (n,e,C) f32, row_slot (n,))}, rows sorted by slot."""
    out = {}
    if len(ex_slot) == 0:
        return out
    row_id = ex_rank // EMAX
    within = ex_rank % EMAX
    key = ex_slot * 64 + row_id
    ukey, uinv = np.unique(key, return_inverse=True)
    u_slot = ukey // 64
    row_n = np.bincount(uinv)
    barr = np.asarray(EBUCKETS)
    row_e = barr[np.searchsorted(barr, row_n)]
    for e in EBUCKETS:
        rows_mask = row_e == e
        nrows = int(rows_mask.sum())
        if nrows == 0:
            continue
        ridx = np.nonzero(rows_mask)[0]
        rmap = np.full(len(ukey), -1, np.int64)
        rmap[ridx] = np.arange(nrows)
        cmask = rmap[uinv] >= 0
        rows = np.tile(inert_row.astype(np.float32), (nrows, e, 1))
        rows[rmap[uinv[cmask]], within[cmask]] = ex_vals[cmask]
        out[e] = (rows, u_slot[ridx])
    return out


def _chunk_table(per_core_rows, chk_entries):
    """Global chunk list [(e, n_pad)] with n_pad rows <= 128*(chk//e),
    plus per-core per-chunk (rows, slots)."""
    table = []
    core_chunks = [[] for _ in per_core_rows]
    for e in EBUCKETS:
        nmax = max((len(rc[e][1]) if e in rc else 0) for rc in per_core_rows)
        if nmax == 0:
            continue
        n_pad_total = ((nmax + 127) // 128) * 128
        cap = 128 * (chk_entries // e)
        start = 0
        while start < n_pad_total:
            n_pad = min(cap, n_pad_total - start)
            table.append((e, n_pad))
            for ci, rc in enumerate(per_core_rows):
                rows, slots = rc.get(e, (np.zeros((0, e, 0), np.float32),
                                         np.zeros(0, np.int64)))
                core_chunks[ci].append((rows[start:start + n_pad],
                                        slots[start:start + n_pad]))
            start += n_pad
    return table, core_chunks


def _sec_device_layout(rows, n_pad, e, inert_row):
    """(n, e, C) rows -> flat (128*C*rpp*e): row q -> partition q%128,
    row-slot q//128; per-partition comp-major."""
    C = len(inert_row)
    n = rows.shape[0]
    full = np.tile(inert_row.astype(np.float32), (n_pad, e, 1))
    if n:
        full[:n] = rows
    rpp = n_pad // 128
    arr = full.reshape(rpp, 128, e, C).transpose(1, 3, 0, 2)
    return np.ascontiguousarray(arr).reshape(-1)


def _prepare(inputs):
    atom_index = np.asarray(inputs["atom_index"])
    pair_indices = np.asarray(inputs["pair_indices"])
    d_ij = np.asarray(inputs["d_ij"])
    r_ij = np.asarray(inputs["r_ij"])
    central = np.asarray(inputs["central_atom_index"])
    p12 = np.asarray(inputs["pair_index12"])
    sign12 = np.asarray(inputs["sign12"])

    i, j = pair_indices[0], pair_indices[1]
    si, sj = atom_index[i], atom_index[j]
    d = d_ij[:, 0].astype(np.float32)

    dest = np.concatenate([i, j])
    osp = np.concatenate([sj, si]).astype(np.int64)
    dval = np.concatenate([d, d]).astype(np.float32)[:, None]

    p0, p1 = p12[0], p12[1]
    v0 = r_ij[p0] * sign12[0].astype(np.float32)[:, None]
    v1 = r_ij[p1] * sign12[1].astype(np.float32)[:, None]
    d0 = d[p0]
    d1 = d[p1]
    mu = 0.95 * np.einsum("ij,ij->i", v0, v1) / (d0 * d1)
    s0 = np.where(sign12[0] == 1, sj[p0], si[p0])
    s1 = np.where(sign12[1] == 1, sj[p1], si[p1])
    cls = TRIU[s0, s1].astype(np.int64)
    geom = np.stack([mu, d0, d1], axis=1).astype(np.float32)

    inert_r = np.array([INERT_D_R], np.float32)
    dense_cores = []
    rows_r_cores = []
    rows_a_cores = []
    for c in range(NCORE):
        base = c * NB
        m = (dest >= base) & (dest < base + NB)
        slot_r = ((dest[m] - base) * S + osp[m]).astype(np.int64)
        dense_r, ex_r = _plan_dense(slot_r, dval[m], RSLOTS_P, ELL_R,
                                    BLOCKS_R, inert_r)
        rows_r_cores.append(_plan_rows(*ex_r, inert_r))

        m = (central >= base) & (central < base + NB)
        slot_a = ((central[m] - base) * NPAIRS + cls[m]).astype(np.int64)
        dense_a, ex_a = _plan_dense(slot_a, geom[m], ASLOTS_P, ELL_A,
                                    BLOCKS_A, INERT_A)
        rows_a_cores.append(_plan_rows(*ex_a, INERT_A))
        dense_cores.append((dense_r, dense_a))

    table_r, chunks_r = _chunk_table(rows_r_cores, E_R)
    table_a, chunks_a = _chunk_table(rows_a_cores, E_A)

    in_maps = []
    merge = []
    for c in range(NCORE):
        dense_r, dense_a = dense_cores[c]
        # mu: global p-major (128, PPT_A); d01: per block [p][comp][E]
        mu_cols = []
        d01_parts = []
        for (off, P_, M_, E, e0) in META_A:
            a = dense_a[off * ELL_A:(off + P_ * M_) * ELL_A].reshape(P_, E, 3)
            mu_cols.append(a[:, :, 0])
            d01_parts.append(np.ascontiguousarray(
                a[:, :, 1:3].transpose(0, 2, 1)).reshape(-1))
        mu_flat = np.ascontiguousarray(
            np.concatenate(mu_cols, axis=1)).reshape(-1)
        d01 = np.concatenate(d01_parts)
        dr_parts = []
        for (off, P_, M_, E, e0) in META_R:
            a = dense_r[off * ELL_R:(off + P_ * M_) * ELL_R].reshape(P_, E)
            dr_parts.append(np.ascontiguousarray(a).reshape(-1))
        dr = np.concatenate(dr_parts)    # block-major [blk][p][E]

        gext_parts = []
        mrg_a = []
        for ti, (e, n_pad) in enumerate(table_a):
            rows, rslot = chunks_a[c][ti]
            gext_parts.append(_sec_device_layout(rows, n_pad, e, INERT_A))
            mrg_a.append(rslot)
        drext_parts = []
        mrg_r = []
        for ti, (e, n_pad) in enumerate(table_r):
            rows, rslot = chunks_r[c][ti]
            drext_parts.append(_sec_device_layout(rows, n_pad, e, inert_r))
            mrg_r.append(rslot)

        in_maps.append({
            "mu": mu_flat,
            "d01": d01,
            "dr": dr,
            "gext": (np.concatenate(gext_parts) if gext_parts
                     else np.zeros(128, np.float32)),
            "drext": (np.concatenate(drext_parts) if drext_parts
                      else np.zeros(128, np.float32)),
        })
        merge.append((mrg_r, mrg_a))

    layout = dict(
        table_r=table_r, table_a=table_a,
        gext_len=max(1, sum(3 * (n // 128) * e for (e, n) in table_a)),
        drext_len=max(1, sum((n // 128) * e for (e, n) in table_r)),
        ext_r_len=max(1, sum((n // 128) * NRBF for (e, n) in table_r)),
        ext_a_len=max(1, sum((n // 128) * SUB for (e, n) in table_a)),
    )
    return in_maps, layout, merge


# --------------------------------------------------------------------------
# Device kernel builder
# --------------------------------------------------------------------------


def build_nc(layout):
    nc = bass.Bass()
    mu_p = nc.declare_dram_parameter("mu", [128 * PPT_A], F32, isOutput=False)
    d01_p = nc.declare_dram_parameter("d01", [128 * 2 * PPT_A], F32,
                                      isOutput=False)
    dr_p = nc.declare_dram_parameter("dr", [128 * PPT_R], F32, isOutput=False)
    gext_p = nc.declare_dram_parameter("gext", [128 * layout["gext_len"]],
                                       F32, isOutput=False)
    drext_p = nc.declare_dram_parameter("drext", [128 * layout["drext_len"]],
                                        F32, isOutput=False)
    out_r = nc.declare_dram_parameter("out_r", [RSLOTS_P * NRBF], BF16,
                                      isOutput=True)
    out_a = nc.declare_dram_parameter("out_a", [ASLOTS_P * SUB], BF16,
                                      isOutput=True)
    ext_r = nc.declare_dram_parameter("ext_r", [128 * layout["ext_r_len"]],
                                      BF16, isOutput=True)
    ext_a = nc.declare_dram_parameter("ext_a", [128 * layout["ext_a_len"]],
                                      BF16, isOutput=True)

    bias_vals = [HALFPI, 1.0, -float(CENTERS_R[0]), -float(SHFA[0]),
                 SHIFT_A, -SHIFT_A]
    for k, v in enumerate(sorted(set(bias_vals))):
        t = nc.alloc_sbuf_tensor(f"bconst{k}", [128, 1], F32)
        nc.gpsimd.memset(t.ap(), v)
        nc.const_aps.aps[(F32, v)] = t.ap()
    nc.all_engine_barrier()

    act = nc.scalar.activation
    vec = nc.vector
    gps = nc.gpsimd
    tre = vec if os.environ.get("ANI_TREES", "vec") == "vec" else gps

    with TileContext(nc) as tc:
        with tc.tile_pool(name="main", bufs=1) as pool:
            # ---------------- static loads ----------------
            mu_t = pool.tile([128, PPT_A], F32, tag="mu", name="mu")
            nc.sync.dma_start(out=mu_t[:, :],
                              in_=mu_p[:].rearrange("(p q) -> p q", p=128))
            ga_secs = []
            off = 0
            for ti, (e, n_pad) in enumerate(layout["table_a"]):
                rpp = n_pad // 128
                t = pool.tile([128, 3, rpp, e], F32, tag=f"gae{ti}",
                              name=f"gae{ti}")
                nc.sync.dma_start(
                    out=t[:, :, :, :].rearrange("p a b c -> p (a b c)"),
                    in_=gext_p[off:off + 128 * 3 * rpp * e]
                    .rearrange("(p q) -> p q", p=128))
                ga_secs.append(t)
                off += 128 * 3 * rpp * e
            dr_secs = []
            off = 0
            for ti, (e, n_pad) in enumerate(layout["table_r"]):
                rpp = n_pad // 128
                t = pool.tile([128, rpp * e], F32, tag=f"dre{ti}",
                              name=f"dre{ti}")
                nc.sync.dma_start(
                    out=t[:, :],
                    in_=drext_p[off:off + 128 * rpp * e]
                    .rearrange("(p q) -> p q", p=128))
                dr_secs.append(t)
                off += 128 * rpp * e

            # ---------------- static derived ----------------
            prod_t = pool.tile([128, PPT_A], BF16, tag="prod", name="prod")
            davs_t = pool.tile([128, PPT_A], BF16, tag="davs", name="davs")
            sig_t = pool.tile([128, PPT_A], BF16, tag="sig", name="sig")
            fc2_t = pool.tile([128, PPT_A], BF16, tag="fc2", name="fc2")
            ba_t = pool.tile([128, PPT_A], BF16, tag="ba", name="ba")
            f20_t = pool.tile([128, PPT_A], BF16, tag="f20", name="f20")
            fcs_t = pool.tile([128, PPT_R], BF16, tag="fcs", name="fcs")

            # ============ PHASE A: all Sin-table work ============
            for (off_, P_, M_, E, e0) in META_A:
                d01b = pool.tile([128, 2 * E_A], F32, tag="d01", name="d01",
                                 bufs=2)
                base = 128 * 2 * e0
                nc.sync.dma_start(
                    out=d01b[:, 0:2 * E],
                    in_=d01_p[base:base + 128 * 2 * E]
                    .rearrange("(p q) -> p q", p=128))
                s0 = pool.tile([128, E_A], BF16, tag="s0", name="s0", bufs=2)
                s1 = pool.tile([128, E_A], BF16, tag="s1", name="s1", bufs=2)
                act(s0[:, 0:E], d01b[:, 0:E], AF.Sin, scale=SIN_SCALE_A,
                    bias=HALFPI)
                act(s1[:, 0:E], d01b[:, E:2 * E], AF.Sin, scale=SIN_SCALE_A,
                    bias=HALFPI)
                vec.tensor_tensor(prod_t[:, e0:e0 + E], s0[:, 0:E],
                                  s1[:, 0:E], OP.mult)
                vec.tensor_tensor(davs_t[:, e0:e0 + E], d01b[:, 0:E],
                                  d01b[:, E:2 * E], OP.add)
            for (off_, P_, M_, E, e0) in META_R:
                drb = pool.tile([128, E_R], F32, tag="drA", name="drA",
                                bufs=2)
                nc.sync.dma_start(
                    out=drb[:, 0:E],
                    in_=dr_p[128 * e0:128 * (e0 + E)]
                    .rearrange("(p q) -> p q", p=128))
                act(fcs_t[:, e0:e0 + E], drb[:, 0:E], AF.Sin,
                    scale=SIN_SCALE_R, bias=HALFPI)
            ga_A = []
            for ti, (e, n_pad) in enumerate(layout["table_a"]):
                rpp = n_pad // 128
                Ein = rpp * e
                ga = ga_secs[ti]
                s0 = pool.tile([128, E_A], BF16, tag="s0", name="s0e",
                               bufs=2)
                s1 = pool.tile([128, E_A], BF16, tag="s1", name="s1e",
                               bufs=2)
                d0v = ga[:, 1, :, :].rearrange("p a b -> p (a b)")
                d1v = ga[:, 2, :, :].rearrange("p a b -> p (a b)")
                act(s0[:, 0:Ein], d0v, AF.Sin, scale=SIN_SCALE_A,
                    bias=HALFPI)
                act(s1[:, 0:Ein], d1v, AF.Sin, scale=SIN_SCALE_A,
                    bias=HALFPI)
                prode = pool.tile([128, Ein], BF16, tag=f"prE{ti}",
                                  name=f"prE{ti}")
                davse = pool.tile([128, Ein], BF16, tag=f"dvE{ti}",
                                  name=f"dvE{ti}")
                vec.tensor_tensor(prode[:, :], s0[:, 0:Ein], s1[:, 0:Ein],
                                  OP.mult)
                vec.tensor_tensor(davse[:, :], d0v, d1v, OP.add)
                ga_A.append((prode, davse))
            fcs_secs = []
            for ti, (e, n_pad) in enumerate(layout["table_r"]):
                rpp = n_pad // 128
                t = pool.tile([128, rpp * e], BF16, tag=f"fcE{ti}",
                              name=f"fcE{ti}")
                act(t[:, :], dr_secs[ti][:, :], AF.Sin, scale=SIN_SCALE_R,
                    bias=HALFPI)
                fcs_secs.append(t)

            # ============ batched B-prep (exp/ln table) ============
            def b_prep(mu_ap, prod_ap, davs_ap, sig_ap, fc2_ap, ba_ap,
                       f20_ap, n, tmptag, nametag):
                tmp1 = pool.tile([128, n], F32, tag=tmptag,
                                 name=nametag + "t1", bufs=2)
                tmp2 = pool.tile([128, n], F32, tag=tmptag,
                                 name=nametag + "t2", bufs=2)
                act(tmp1[:, :], mu_ap, AF.Square)
                act(tmp2[:, :], tmp1[:, :], AF.Ln, scale=-1.0, bias=1.0)
                act(sig_ap, tmp2[:, :], AF.Exp, scale=0.5)
                act(fc2_ap, prod_ap, AF.Square, scale=float(np.sqrt(2.0)))
                act(ba_ap, davs_ap, AF.Exp, scale=B_A_SCALE)
                tmp3 = pool.tile([128, n], F32, tag=tmptag,
                                 name=nametag + "t3", bufs=2)
                act(tmp3[:, :], davs_ap, AF.Square, scale=0.5,
                    bias=-float(SHFA[0]))
                # f2_0 * e^SHIFT_A (the shift is cancelled inside tjf)
                act(f20_ap, tmp3[:, :], AF.Exp, scale=-ETA_A, bias=SHIFT_A)

            HPPT = (PPT_A + 1) // 2
            for ci, (a, b) in enumerate([(0, HPPT), (HPPT, PPT_A)]):
                sl = slice(a, b)
                b_prep(mu_t[:, sl], prod_t[:, sl], davs_t[:, sl],
                       sig_t[:, sl], fc2_t[:, sl], ba_t[:, sl],
                       f20_t[:, sl], b - a, "btmp", f"bp{ci}")
            ext_B = []
            for ti, (e, n_pad) in enumerate(layout["table_a"]):
                rpp = n_pad // 128
                Ein = rpp * e
                prode, davse = ga_A[ti]
                sige = pool.tile([128, Ein], BF16, tag=f"sgE{ti}",
                                 name=f"sgE{ti}")
                fc2e = pool.tile([128, Ein], BF16, tag=f"fcE2{ti}",
                                 name=f"fcE2{ti}")
                bae = pool.tile([128, Ein], BF16, tag=f"baE{ti}",
                                name=f"baE{ti}")
                f20e = pool.tile([128, Ein], BF16, tag=f"f20E{ti}",
                                 name=f"f20E{ti}")
                mue = ga_secs[ti][:, 0, :, :].rearrange("p a b -> p (a b)")
                b_prep(mue, prode[:, :], davse[:, :], sige[:, :],
                       fc2e[:, :], bae[:, :], f20e[:, :], Ein, "betmp",
                       f"bpe{ti}")
                ext_B.append((sige, fc2e, bae, f20e))

            # ============ angular term pipeline (wj-major) ============
            def angular_terms3(mu_ap, sig_ap, fc2_ap, ba_ap, f20_ap, E,
                               terms3, nm):
                """APs are (128, E) views; terms3 is a [128, SUB, E] view."""
                tj = pool.tile([128, NZ, E_A], F32, tag="tj", name="tj" + nm,
                               bufs=1)
                for jj in range(NZ):
                    vec.tensor_scalar(tj[:, jj, 0:E], sig_ap,
                                      0.5 * float(SINZ[jj]), 0.5,
                                      OP.mult, OP.add)
                    vec.scalar_tensor_tensor(tj[:, jj, 0:E], mu_ap,
                                             0.5 * float(COSZ[jj]),
                                             tj[:, jj, 0:E], OP.mult,
                                             OP.add)
                vec.tensor_scalar(tj[:, :, 0:E], tj[:, :, 0:E], 1e-20,
                                  None, OP.max, OP.bypass)
                act(tj[:, :, 0:E], tj[:, :, 0:E], AF.Ln)
                tjf = pool.tile([128, NZ, E_A], BF16, tag="tjf",
                                name="tjf" + nm, bufs=2)
                act(tjf[:, :, 0:E], tj[:, :, 0:E], AF.Exp, scale=ZETA,
                    bias=-SHIFT_A)
                for jj in range(NZ):
                    vec.tensor_tensor(tjf[:, jj, 0:E], tjf[:, jj, 0:E],
                                      fc2_ap, OP.mult)

                f2 = pool.tile([128, NA, E_A], BF16, tag="f2",
                               name="f2" + nm, bufs=1)
                vec.tensor_copy(f2[:, 0, 0:E], f20_ap)
                for w in range(1, NA):
                    vec.scalar_tensor_tensor(f2[:, w, 0:E], ba_ap,
                                             K_A[w - 1], f2[:, w - 1, 0:E],
                                             OP.mult, OP.mult)
                o4 = terms3.rearrange("p (w j) e -> p w j e", w=NA)
                f2v = f2[:, :, 0:E].unsqueeze(2) \
                    .broadcast_to([128, NA, NZ, E])
                tjv = tjf[:, :, 0:E].unsqueeze(1) \
                    .broadcast_to([128, NA, NZ, E])
                vec.tensor_tensor(o4, f2v, tjv, OP.mult)

            def angular_block(bi):
                (off_, P_, M_, E, e0) = META_A[bi]
                terms3 = pool.tile([128, SUB, E_A], BF16, tag="t24",
                                   name="t3", bufs=2)
                sl = slice(e0, e0 + E)
                angular_terms3(mu_t[:, sl], sig_t[:, sl], fc2_t[:, sl],
                               ba_t[:, sl], f20_t[:, sl], E,
                               terms3[:, :, 0:E], f"d{bi}")
                dst = out_a[off_ * SUB:(off_ + P_ * M_) * SUB] \
                    .rearrange("(p q) -> p q", p=P_)
                nc.sync.dma_start(out=dst, in_=terms3[:P_, :, 0:M_])

            # ============ radial term pipeline (log-space chain) ========
            def radial_block(bi):
                (off_, P_, M_, E, e0) = META_R[bi]
                drb = pool.tile([128, E_R], F32, tag="drB", name="drB",
                                bufs=2)
                nc.sync.dma_start(
                    out=drb[:, 0:E],
                    in_=dr_p[128 * e0:128 * (e0 + E)]
                    .rearrange("(p q) -> p q", p=128))
                fin16 = pool.tile([128, M_R * NRBF], BF16, tag="rfin",
                                  name="rfin", bufs=2)
                radial_terms(drb[:, 0:E], fcs_t[:, e0:e0 + E], E, M_,
                             fin16, f"rd{bi}")
                dst = out_r[off_ * NRBF:(off_ + P_ * M_) * NRBF] \
                    .rearrange("(p q) -> p q", p=P_)
                f16v = fin16[:, :].rearrange("p (a b) -> p a b", a=NRBF)
                nc.sync.dma_start(out=dst, in_=f16v[:P_, :, 0:M_])

            def radial_terms(d_ap, fcs_ap, E, M_, fin16, nm):
                """Dense radial: E = 8*M_ entries -> fin16[:, 0:16*M_]
                in r-major layout [p, r, m] (host transposes)."""
                sq = pool.tile([128, E_R], F32, tag="rt0", name="rt0" + nm,
                               bufs=2)
                act(sq[:, 0:E], d_ap, AF.Square, bias=-float(CENTERS_R[0]))
                lnfc = pool.tile([128, E_R], F32, tag="rfc", name="rfc" + nm,
                                 bufs=2)
                act(lnfc[:, 0:E], fcs_ap, AF.Square, scale=0.5)
                act(lnfc[:, 0:E], lnfc[:, 0:E], AF.Ln)
                vv = pool.tile([128, E_R], F32, tag="rbb", name="rbb" + nm,
                               bufs=2)
                vec.tensor_scalar(vv[:, 0:E], d_ap, B_R_SCALE, CV_R,
                                  OP.mult, OP.add)
                logt = pool.tile([128, NRBF, E_R], F32, tag="t24",
                                 name="rlog" + nm, bufs=2)
                vec.scalar_tensor_tensor(logt[:, 0, 0:E], sq[:, 0:E],
                                         -ETA_R, lnfc[:, 0:E], OP.mult,
                                         OP.add)
                for r in range(1, NRBF):
                    eng = vec if r < CHAIN_SPLIT else gps
                    eng.scalar_tensor_tensor(logt[:, r, 0:E], vv[:, 0:E],
                                             KL_R[r - 1],
                                             logt[:, r - 1, 0:E],
                                             OP.add, OP.add)
                terms = pool.tile([128, NRBF, E_R], BF16, tag="rterms",
                                  name="rterms" + nm, bufs=2)
                act(terms[:, :, 0:E], logt[:, :, 0:E], AF.Exp)
                M4 = E // 2
                t4 = pool.tile([128, NRBF, E_R // 2], BF16, tag="rt4",
                               name="rt4" + nm, bufs=1)
                vec.tensor_tensor(t4[:, :, 0:M4], terms[:, :, 0:M4],
                                  terms[:, :, M4:E], OP.add)
                t2t = pool.tile([128, NRBF, E_R // 4], BF16, tag="rt2",
                                name="rt2" + nm, bufs=2)
                tre.tensor_tensor(t2t[:, :, 0:M4 // 2], t4[:, :, 0:M4 // 2],
                                  t4[:, :, M4 // 2:M4], OP.add)
                f16v = fin16[:, :].rearrange("p (a b) -> p a b", a=NRBF)
                tre.tensor_tensor(f16v[:, :, 0:M_], t2t[:, :, 0:M_],
                                  t2t[:, :, M_:2 * M_], OP.add)

            # ---- merged main loop: interleave angular + radial blocks ----
            nA, nR = len(META_A), len(META_R)
            orderA = [("A", i) for i in range(nA)]
            orderR = [("R", i) for i in range(nR)]
            merged = []
            fa = fr = 0
            while fa < nA or fr < nR:
                if fa < nA and (fr >= nR or fa * nR <= fr * nA):
                    merged.append(orderA[fa])
                    fa += 1
                else:
                    merged.append(orderR[fr])
                    fr += 1
            for kind, bi in merged:
                if kind == "A":
                    angular_block(bi)
                else:
                    radial_block(bi)

            # ============ extras: angular chunks ============
            eoff = 0
            for ti, (e, n_pad) in enumerate(layout["table_a"]):
                rpp = n_pad // 128
                Ein = rpp * e
                sige, fc2e, bae, f20e = ext_B[ti]
                mue = ga_secs[ti][:, 0, :, :].rearrange("p a b -> p (a b)")
                terms3 = pool.tile([128, SUB, E_A], BF16, tag="t24",
                                   name=f"t3E{ti}", bufs=2)
                angular_terms3(mue, sige[:, :], fc2e[:, :], bae[:, :],
                               f20e[:, :], Ein, terms3[:, :, 0:Ein],
                               f"e{ti}")
                if e == 1:
                    src = terms3[:, :, 0:rpp]
                else:
                    tv = terms3[:, :, 0:Ein].rearrange(
                        "p s (a b) -> p s a b", b=e)
                    ee = e
                    while ee > 2:
                        tre.tensor_tensor(tv[:, :, :, 0:ee // 2],
                                          tv[:, :, :, 0:ee // 2],
                                          tv[:, :, :, ee // 2:ee], OP.add)
                        ee //= 2
                    asum = pool.tile([128, SUB, rpp], BF16, tag="asum",
                                     name=f"asum{ti}", bufs=2)
                    tre.tensor_tensor(asum[:, :, :], tv[:, :, :, 0],
                                      tv[:, :, :, 1], OP.add)
                    src = asum[:, :, :]
                nc.sync.dma_start(
                    out=ext_a[128 * eoff:128 * (eoff + rpp * SUB)]
                    .rearrange("(p q) -> p q", p=128),
                    in_=src)
                eoff += rpp * SUB

            # ============ extras: radial chunks ============
            eoff = 0
            for ti, (e, n_pad) in enumerate(layout["table_r"]):
                rpp = n_pad // 128
                Ein = rpp * e
                sq = pool.tile([128, E_R], F32, tag="rt0", name=f"sqE{ti}",
                               bufs=2)
                act(sq[:, 0:Ein], dr_secs[ti][:, :], AF.Square,
                    bias=-float(CENTERS_R[0]))
                lnfc = pool.tile([128, E_R], F32, tag="rfc",
                                 name=f"lfE{ti}", bufs=2)
                act(lnfc[:, 0:Ein], fcs_secs[ti][:, :], AF.Square, scale=0.5)
                act(lnfc[:, 0:Ein], lnfc[:, 0:Ein], AF.Ln)
                vv = pool.tile([128, E_R], F32, tag="rbb", name=f"vvE{ti}",
                               bufs=2)
                vec.tensor_scalar(vv[:, 0:Ein], dr_secs[ti][:, :],
                                  B_R_SCALE, CV_R, OP.mult, OP.add)
                logt = pool.tile([128, NRBF, E_R], F32, tag="t24",
                                 name=f"lRE{ti}", bufs=2)
                vec.scalar_tensor_tensor(logt[:, 0, 0:Ein], sq[:, 0:Ein],
                                         -ETA_R, lnfc[:, 0:Ein], OP.mult,
                                         OP.add)
                for r in range(1, NRBF):
                    eng = vec if r < CHAIN_SPLIT else gps
                    eng.scalar_tensor_tensor(logt[:, r, 0:Ein], vv[:, 0:Ein],
                                             KL_R[r - 1],
                                             logt[:, r - 1, 0:Ein],
                                             OP.add, OP.add)
                terms = pool.tile([128, NRBF, E_R], BF16, tag="rterms",
                                  name=f"tRE{ti}", bufs=2)
                act(terms[:, :, 0:Ein], logt[:, :, 0:Ein], AF.Exp)
                if e == 1:
                    src = terms[:, :, 0:rpp]
                else:
                    tv = terms[:, :, 0:Ein].rearrange(
                        "p c (a b) -> p c a b", b=e)
                    ee = e
                    while ee > 2:
                        tre.tensor_tensor(tv[:, :, :, 0:ee // 2],
                                          tv[:, :, :, 0:ee // 2],
                                          tv[:, :, :, ee // 2:ee], OP.add)
                        ee //= 2
                    rsum = pool.tile([128, NRBF, rpp], BF16, tag="rsum",
                                     name=f"rsum{ti}", bufs=2)
                    tre.tensor_tensor(rsum[:, :, :], tv[:, :, :, 0],
                                      tv[:, :, :, 1], OP.add)
                    src = rsum[:, :, :]
                nc.sync.dma_start(
                    out=ext_r[128 * eoff:128 * (eoff + rpp * NRBF)]
                    .rearrange("(p q) -> p q", p=128),
                    in_=src)
                eoff += rpp * NRBF

    lower_extended_insts(nc)
    _split_excess_waits(nc, 1)
    return nc


def _split_excess_waits(nc, max_waits=1):
    """This neuronxcc build rejects >1 sem-wait per instruction at codegen;
    hoist extras onto preceding event-semaphore carriers."""
    for f in nc.m.functions:
        for b in f.blocks:
            idx = 0
            while idx < len(b.instructions):
                inst = b.instructions[idx]
                si = inst.sync_info
                if si is not None and len(si.on_wait) > max_waits:
                    waits = list(si.on_wait)
                    keep = waits[-max_waits:]
                    head = waits[:-max_waits]
                    at = idx
                    for i0 in range(0, len(head), max_waits):
                        chunk = head[i0:i0 + max_waits]
                        ev = mybir.InstEventSemaphore(
                            name=nc.get_next_instruction_name(), ins=[],
                            outs=[])
                        ev.engine = inst.engine
                        ev.sync_info = mybir.SyncInfo(on_wait=chunk,
                                                      on_update=[])
                        nc.register_instruction(ev)
                        b.instructions.insert(at, ev)
                        at += 1
                        idx += 1
                    si.on_wait = keep
                    inst.sync_info = si
                idx += 1


# --------------------------------------------------------------------------
# Entry point
# --------------------------------------------------------------------------

LAST_RESULT = {}


def kernel(**inputs):
    in_maps, layout, merge = _prepare(inputs)
    nc = build_nc(layout)
    trace = os.environ.get("ANI_TRACE") == "1"
    res = run_bass_kernel_spmd(nc, in_maps, core_ids=list(range(NCORE)),
                               trace=trace)
    LAST_RESULT["exec_time_ns"] = getattr(res, "exec_time_ns", None)
    LAST_RESULT["res"] = res

    parts = []
    for c in range(NCORE):
        rad_raw = np.asarray(res.results[c]["out_r"]).astype(np.float32)
        rad = np.empty((RSLOTS_P, NRBF), np.float32)
        for (off, P_, M_, E, e0) in META_R:
            seg = rad_raw[off * NRBF:(off + P_ * M_) * NRBF] \
                .reshape(P_, NRBF, M_).transpose(0, 2, 1)
            rad[off:off + P_ * M_] = seg.reshape(P_ * M_, NRBF)
        ang_raw = np.asarray(res.results[c]["out_a"]).astype(np.float32)
        ang = np.empty((ASLOTS_P, SUB), np.float32)
        for (off, P_, M_, E, e0) in META_A:
            seg = ang_raw[off * SUB:(off + P_ * M_) * SUB] \
                .reshape(P_, SUB, M_).transpose(0, 2, 1)
            ang[off:off + P_ * M_] = seg.reshape(P_ * M_, SUB)
        er = np.asarray(res.results[c]["ext_r"]).astype(np.float32)
        ea = np.asarray(res.results[c]["ext_a"]).astype(np.float32)
        mrg_r, mrg_a = merge[c]
        eoff = 0
        for ti, (e, n_pad) in enumerate(layout["table_r"]):
            rpp = n_pad // 128
            sums = er[128 * eoff:128 * (eoff + rpp * NRBF)] \
                .reshape(128, NRBF, rpp).transpose(0, 2, 1)
            slots = mrg_r[ti]
            if len(slots):
                q = np.arange(len(slots))
                np.add.at(rad, slots, sums[q % 128, q // 128])
            eoff += rpp * NRBF
        eoff = 0
        for ti, (e, n_pad) in enumerate(layout["table_a"]):
            rpp = n_pad // 128
            sums = ea[128 * eoff:128 * (eoff + rpp * SUB)] \
                .reshape(128, SUB, rpp).transpose(0, 2, 1)
            slots = mrg_a[ti]
            if len(slots):
                q = np.arange(len(slots))
                np.add.at(ang, slots, sums[q % 128, q // 128])
            eoff += rpp * SUB
        parts.append(np.concatenate(
            [rad[:RSLOTS].reshape(NB, S * NRBF),
             ang[:ASLOTS].reshape(NB, NPAIRS * SUB)], axis=1))
    return np.concatenate(parts, axis=0).astype(np.float32)


# revision 29
# speedup vs baseline: 2.3931x; 1.1493x over previous
"""ANI AEV representation kernel for 8 Trainium2 NeuronCores — v3.

Design (data-parallel over atoms, per the sharding hint):
  - Atoms are partitioned into 8 contiguous shards of 6250; each core
    computes its (6250, 1008) AEV slice.
  - Angular: ELL=1 dense layout (one contribution per slot; overflow goes
    through bucketed extra rows summed on device, merged on host).
    Terms are produced in (w,j)-major layout [p, 32, e] so the 8x4 outer
    product is ONE contiguous bf16 tensor_tensor at 2x DVE mode; the host
    permutes (wj, m) -> (m, wj) while unsharding.
  - Radial: ELL=8 dense layout, 48 slots/partition/block (384 entries) so
    the 15-step log-space recurrence amortizes instruction overhead; the
    recurrence tail + reduction tree run on GpSimd to unload the DVE.
  - All Sin evaluations batched in one phase (one ACT table switch);
    exp/ln set stays loaded for the rest.
  - Outputs are bf16 (halves HBM write traffic); host upcasts.
"""

import os
import sys

sys.path.insert(0, "/opt/trn_rl_repo")

import numpy as np

import concourse.bass as bass
import concourse.mybir as mybir
from concourse.library_overlay import lower_extended_insts
from concourse.bass_utils import run_bass_kernel_spmd
from concourse.tile import TileContext

# ---- problem constants (must match reference.py) ----
N = 50000
NCORE = 8
NB = N // NCORE          # 6250 atoms per core
S = 7
NRBF = 16
RC = 0.51
RMIN = 0.08
RCA = 0.35
RAMIN = 0.08
NA = 8
NZ = 4
ETA_R = 1970.0
ETA_A = 1250.0
ZETA = 14.1
NPAIRS = S * (S + 1) // 2   # 28
SUB = NA * NZ               # 32

RSLOTS = NB * S             # 43750 radial slots per core
ASLOTS = NB * NPAIRS        # 175000 angular slots per core

ELL_R = 6
ELL_A = 1
M_R = 64                    # radial slots per partition per block
M_A = 384                   # angular slots per partition per block
E_R = ELL_R * M_R           # 384 entries/partition per radial block
E_A = ELL_A * M_A           # 384 entries/partition per angular block

# gpsimd cannot run scalar_tensor_tensor (walrus backend crash); 16 keeps
# the whole radial recurrence on the DVE.
CHAIN_SPLIT = int(os.environ.get("ANI_CHAIN_SPLIT", "16"))

EBUCKETS = (1, 2, 4, 8, 16)
EMAX = EBUCKETS[-1]

F32 = mybir.dt.float32
BF16 = mybir.dt.bfloat16
AF = mybir.ActivationFunctionType
OP = mybir.AluOpType

CENTERS_R = (RMIN + (RC - RMIN) / NRBF * np.arange(NRBF)).astype(np.float64)
DLT_R = (RC - RMIN) / NRBF
SHFA = (RAMIN + (RCA - RAMIN) / NA * np.arange(NA)).astype(np.float64)
DLT_A = (RCA - RAMIN) / NA
SHFZ = ((np.arange(NZ) + 0.5) * (np.pi / NZ)).astype(np.float64)
COSZ = np.cos(SHFZ)
SINZ = np.sin(SHFZ)

# radial log-space chain: log t_r = log t_{r-1} + v + KL_r with
# v = 2*eta*dlt*(d - c0), KL_r = -eta*dlt^2*(2r-1)
B_R_SCALE = float(2.0 * ETA_R * DLT_R)
CV_R = float(-2.0 * ETA_R * DLT_R * CENTERS_R[0])
KL_R = [float(-ETA_R * DLT_R * DLT_R * (2 * r - 1)) for r in range(1, NRBF)]
# angular f2 multiplicative chain, shifted by e^SHIFT_A to stay normal:
# f2s_w = f2s_{w-1} * Ba * K_w, Ba = exp(eta*dlt*davsum)
K_A = [float(np.exp(-ETA_A * DLT_A * (SHFA[w] + SHFA[w - 1])))
       for w in range(1, NA)]
B_A_SCALE = float(ETA_A * DLT_A)   # applied to davsum = d0 + d1
SHIFT_A = 45.0                     # f2 carries e^+S, tjf carries e^-S

SIN_SCALE_R = float(np.pi / (2.0 * RC))
SIN_SCALE_A = float(np.pi / (2.0 * RCA))
HALFPI = float(np.pi / 2.0)

INERT_D_R = 0.7             # pads: B finite, exp(-eta*(d-c)^2) == 0
INERT_A = np.array([0.0, 0.9, 0.9], np.float32)   # [mu, d0, d1]


def _triu_index_np(num_species):
    s1, s2 = np.triu_indices(num_species)
    ret = np.zeros((num_species, num_species), dtype=np.int64)
    ret[s1, s2] = np.arange(len(s1))
    ret[s2, s1] = np.arange(len(s1))
    return ret


TRIU = _triu_index_np(S)

# --------------------------------------------------------------------------
# Host planning
# --------------------------------------------------------------------------


def _blocks(total, m):
    """Full (128, m) blocks; the tail is a (128, ceil(rem/128)) block over
    padded slot space so every block keeps all 128 partitions busy.
    Returns (blocks, padded_total)."""
    out = []
    off = 0
    n_full = total // (128 * m)
    for _ in range(n_full):
        out.append((off, 128, m))
        off += 128 * m
    rem = total - off
    if rem > 0:
        mt = (rem + 127) // 128
        out.append((off, 128, mt))
        off += 128 * mt
    return out, off


BLOCKS_R, RSLOTS_P = _blocks(RSLOTS, M_R)
BLOCKS_A, ASLOTS_P = _blocks(ASLOTS, M_A)


def _block_meta(blocks, ell):
    meta = []
    e0 = 0
    for (off, P_, M_) in blocks:
        E = ell * M_
        meta.append((off, P_, M_, E, e0))
        e0 += E
    return meta


META_R = _block_meta(BLOCKS_R, ELL_R)
META_A = _block_meta(BLOCKS_A, ELL_A)
PPT_R = sum(m[3] for m in META_R)
PPT_A = sum(m[3] for m in META_A)


def _plan_dense(slots, vals, nslots, ell, blocks, inert_row):
    """First `ell` contributions per slot -> block-aware dense array;
    the rest become extras."""
    order = np.argsort(slots, kind="stable")
    ss = slots[order]
    vs = vals[order]
    counts = np.bincount(ss, minlength=nslots)
    starts = np.zeros(nslots + 1, np.int64)
    np.cumsum(counts, out=starts[1:])
    rank = np.arange(len(ss)) - np.repeat(starts[:-1], counts)

    dense = np.tile(inert_row.astype(np.float32), (nslots * ell, 1))
    keep = rank < ell
    offs = np.array([b[0] for b in blocks])
    bidx = np.searchsorted(offs, ss[keep], side="right") - 1
    boff = offs[bidx]
    bM = np.array([b[2] for b in blocks])[bidx]
    p = (ss[keep] - boff) // bM
    mm = (ss[keep] - boff) % bM
    pos = boff * ell + p * (ell * bM) + rank[keep] * bM + mm
    dense[pos] = vs[keep]
    ex = ~keep
    return dense, (ss[ex], (rank[ex] - ell).astype(np.int64), vs[ex])


def _plan_rows(ex_slot, ex_rank, ex_vals, inert_row):
    """Overflow contributions -> rows of <=EMAX entries bucketed by count.
    Returns {e: (rows (n,e,C) f32, row_slot (n,))}, rows sorted by slot."""
    out = {}
    if len(ex_slot) == 0:
        return out
    row_id = ex_rank // EMAX
    within = ex_rank % EMAX
    key = ex_slot * 64 + row_id
    ukey, uinv = np.unique(key, return_inverse=True)
    u_slot = ukey // 64
    row_n = np.bincount(uinv)
    barr = np.asarray(EBUCKETS)
    row_e = barr[np.searchsorted(barr, row_n)]
    for e in EBUCKETS:
        rows_mask = row_e == e
        nrows = int(rows_mask.sum())
        if nrows == 0:
            continue
        ridx = np.nonzero(rows_mask)[0]
        rmap = np.full(len(ukey), -1, np.int64)
        rmap[ridx] = np.arange(nrows)
        cmask = rmap[uinv] >= 0
        rows = np.tile(inert_row.astype(np.float32), (nrows, e, 1))
        rows[rmap[uinv[cmask]], within[cmask]] = ex_vals[cmask]
        out[e] = (rows, u_slot[ridx])
    return out


def _chunk_table(per_core_rows, chk_entries):
    """Global chunk list [(e, n_pad)] with n_pad rows <= 128*(chk//e),
    plus per-core per-chunk (rows, slots)."""
    table = []
    core_chunks = [[] for _ in per_core_rows]
    for e in EBUCKETS:
        nmax = max((len(rc[e][1]) if e in rc else 0) for rc in per_core_rows)
        if nmax == 0:
            continue
        n_pad_total = ((nmax + 127) // 128) * 128
        cap = 128 * (chk_entries // e)
        start = 0
        while start < n_pad_total:
            n_pad = min(cap, n_pad_total - start)
            table.append((e, n_pad))
            for ci, rc in enumerate(per_core_rows):
                rows, slots = rc.get(e, (np.zeros((0, e, 0), np.float32),
                                         np.zeros(0, np.int64)))
                core_chunks[ci].append((rows[start:start + n_pad],
                                        slots[start:start + n_pad]))
            start += n_pad
    return table, core_chunks


def _sec_device_layout(rows, n_pad, e, inert_row):
    """(n, e, C) rows -> flat (128*C*rpp*e): row q -> partition q%128,
    row-slot q//128; per-partition comp-major."""
    C = len(inert_row)
    n = rows.shape[0]
    full = np.tile(inert_row.astype(np.float32), (n_pad, e, 1))
    if n:
        full[:n] = rows
    rpp = n_pad // 128
    arr = full.reshape(rpp, 128, e, C).transpose(1, 3, 0, 2)
    return np.ascontiguousarray(arr).reshape(-1)


def _prepare(inputs):
    atom_index = np.asarray(inputs["atom_index"])
    pair_indices = np.asarray(inputs["pair_indices"])
    d_ij = np.asarray(inputs["d_ij"])
    r_ij = np.asarray(inputs["r_ij"])
    central = np.asarray(inputs["central_atom_index"])
    p12 = np.asarray(inputs["pair_index12"])
    sign12 = np.asarray(inputs["sign12"])

    i, j = pair_indices[0], pair_indices[1]
    si, sj = atom_index[i], atom_index[j]
    d = d_ij[:, 0].astype(np.float32)

    dest = np.concatenate([i, j])
    osp = np.concatenate([sj, si]).astype(np.int64)
    dval = np.concatenate([d, d]).astype(np.float32)[:, None]

    p0, p1 = p12[0], p12[1]
    v0 = r_ij[p0] * sign12[0].astype(np.float32)[:, None]
    v1 = r_ij[p1] * sign12[1].astype(np.float32)[:, None]
    d0 = d[p0]
    d1 = d[p1]
    mu = 0.95 * np.einsum("ij,ij->i", v0, v1) / (d0 * d1)
    s0 = np.where(sign12[0] == 1, sj[p0], si[p0])
    s1 = np.where(sign12[1] == 1, sj[p1], si[p1])
    cls = TRIU[s0, s1].astype(np.int64)
    geom = np.stack([mu, d0, d1], axis=1).astype(np.float32)

    inert_r = np.array([INERT_D_R], np.float32)
    dense_cores = []
    rows_r_cores = []
    rows_a_cores = []
    for c in range(NCORE):
        base = c * NB
        m = (dest >= base) & (dest < base + NB)
        slot_r = ((dest[m] - base) * S + osp[m]).astype(np.int64)
        dense_r, ex_r = _plan_dense(slot_r, dval[m], RSLOTS_P, ELL_R,
                                    BLOCKS_R, inert_r)
        rows_r_cores.append(_plan_rows(*ex_r, inert_r))

        m = (central >= base) & (central < base + NB)
        slot_a = ((central[m] - base) * NPAIRS + cls[m]).astype(np.int64)
        dense_a, ex_a = _plan_dense(slot_a, geom[m], ASLOTS_P, ELL_A,
                                    BLOCKS_A, INERT_A)
        rows_a_cores.append(_plan_rows(*ex_a, INERT_A))
        dense_cores.append((dense_r, dense_a))

    table_r, chunks_r = _chunk_table(rows_r_cores, E_R)
    table_a, chunks_a = _chunk_table(rows_a_cores, E_A)

    in_maps = []
    merge = []
    for c in range(NCORE):
        dense_r, dense_a = dense_cores[c]
        # mu: global p-major (128, PPT_A); d01: per block [p][comp][E]
        mu_cols = []
        d01_parts = []
        for (off, P_, M_, E, e0) in META_A:
            a = dense_a[off * ELL_A:(off + P_ * M_) * ELL_A].reshape(P_, E, 3)
            mu_cols.append(a[:, :, 0])
            d01_parts.append(np.ascontiguousarray(
                a[:, :, 1:3].transpose(0, 2, 1)).reshape(-1))
        mu_flat = np.ascontiguousarray(
            np.concatenate(mu_cols, axis=1)).reshape(-1)
        d01 = np.concatenate(d01_parts)
        dr_parts = []
        for (off, P_, M_, E, e0) in META_R:
            a = dense_r[off * ELL_R:(off + P_ * M_) * ELL_R].reshape(P_, E)
            dr_parts.append(np.ascontiguousarray(a).reshape(-1))
        dr = np.concatenate(dr_parts)    # block-major [blk][p][E]

        gext_parts = []
        mrg_a = []
        for ti, (e, n_pad) in enumerate(table_a):
            rows, rslot = chunks_a[c][ti]
            rpp = n_pad // 128
            gext_parts.append(
                _sec_device_layout(rows, n_pad, e, INERT_A)
                .reshape(128, 3, rpp * e))
            mrg_a.append(rslot)
        drext_parts = []
        mrg_r = []
        for ti, (e, n_pad) in enumerate(table_r):
            rows, rslot = chunks_r[c][ti]
            rpp = n_pad // 128
            drext_parts.append(
                _sec_device_layout(rows, n_pad, e, inert_r)
                .reshape(128, 1, rpp * e))
            mrg_r.append(rslot)

        in_maps.append({
            "mu": mu_flat,
            "d01": d01,
            "dr": dr,
            "gext": (np.ascontiguousarray(
                np.concatenate(gext_parts, axis=2)).reshape(-1)
                if gext_parts else np.zeros(128, np.float32)),
            "drext": (np.ascontiguousarray(
                np.concatenate(drext_parts, axis=2)).reshape(-1)
                if drext_parts else np.zeros(128, np.float32)),
        })
        merge.append((mrg_r, mrg_a))

    def _groups(table, cap):
        """Greedy consecutive chunk groups with sum(rpp*e) <= cap.
        Returns (groups: list[list[chunk idx]], offs: per-chunk EXT offset)."""
        offs = []
        groups = []
        cur = []
        cur_sz = 0
        off = 0
        for ti, (e, n_pad) in enumerate(table):
            sz = (n_pad // 128) * e
            offs.append(off)
            off += sz
            if cur and cur_sz + sz > cap:
                groups.append(cur)
                cur = []
                cur_sz = 0
            cur.append(ti)
            cur_sz += sz
        if cur:
            groups.append(cur)
        return groups, offs

    groups_a, offs_a = _groups(table_a, E_A)
    groups_r, offs_r = _groups(table_r, E_R)

    layout = dict(
        table_r=table_r, table_a=table_a,
        groups_a=groups_a, offs_a=offs_a,
        groups_r=groups_r, offs_r=offs_r,
        ext_a_tot=max(1, sum((n // 128) * e for (e, n) in table_a)),
        ext_r_tot=max(1, sum((n // 128) * e for (e, n) in table_r)),
        ext_r_len=max(1, sum((n // 128) * NRBF for (e, n) in table_r)),
        ext_a_len=max(1, sum((n // 128) * SUB for (e, n) in table_a)),
    )
    return in_maps, layout, merge


# --------------------------------------------------------------------------
# Device kernel builder
# --------------------------------------------------------------------------


def build_nc(layout):
    nc = bass.Bass()
    mu_p = nc.declare_dram_parameter("mu", [128 * PPT_A], F32, isOutput=False)
    d01_p = nc.declare_dram_parameter("d01", [128 * 2 * PPT_A], F32,
                                      isOutput=False)
    dr_p = nc.declare_dram_parameter("dr", [128 * PPT_R], F32, isOutput=False)
    gext_p = nc.declare_dram_parameter("gext", [128 * 3 * layout["ext_a_tot"]],
                                       F32, isOutput=False)
    drext_p = nc.declare_dram_parameter("drext", [128 * layout["ext_r_tot"]],
                                        F32, isOutput=False)
    out_r = nc.declare_dram_parameter("out_r", [RSLOTS_P * NRBF], BF16,
                                      isOutput=True)
    out_a = nc.declare_dram_parameter("out_a", [ASLOTS_P * SUB], BF16,
                                      isOutput=True)
    ext_r = nc.declare_dram_parameter("ext_r", [128 * layout["ext_r_len"]],
                                      BF16, isOutput=True)
    ext_a = nc.declare_dram_parameter("ext_a", [128 * layout["ext_a_len"]],
                                      BF16, isOutput=True)

    bias_vals = [HALFPI, 1.0, -float(CENTERS_R[0]), -float(SHFA[0]),
                 SHIFT_A, -SHIFT_A]
    for k, v in enumerate(sorted(set(bias_vals))):
        t = nc.alloc_sbuf_tensor(f"bconst{k}", [128, 1], F32)
        nc.gpsimd.memset(t.ap(), v)
        nc.const_aps.aps[(F32, v)] = t.ap()
    nc.all_engine_barrier()

    act = nc.scalar.activation
    vec = nc.vector
    gps = nc.gpsimd
    tre = vec if os.environ.get("ANI_TREES", "vec") == "vec" else gps

    with TileContext(nc) as tc:
        with tc.tile_pool(name="main", bufs=1) as pool:
            # ---------------- static loads ----------------
            mu_t = pool.tile([128, PPT_A], F32, tag="mu", name="mu")
            nc.sync.dma_start(out=mu_t[:, :],
                              in_=mu_p[:].rearrange("(p q) -> p q", p=128))
            EXT_A = layout["ext_a_tot"]
            EXT_R = layout["ext_r_tot"]
            ga_all = pool.tile([128, 3, EXT_A], F32, tag="gaall",
                               name="gaall")
            nc.sync.dma_start(
                out=ga_all[:, :, :].rearrange("p a b -> p (a b)"),
                in_=gext_p[:].rearrange("(p q) -> p q", p=128))
            dr_all = pool.tile([128, EXT_R], F32, tag="drall", name="drall")
            nc.sync.dma_start(
                out=dr_all[:, :],
                in_=drext_p[:].rearrange("(p q) -> p q", p=128))

            # ---------------- static derived ----------------
            prod_t = pool.tile([128, PPT_A], BF16, tag="prod", name="prod")
            davs_t = pool.tile([128, PPT_A], BF16, tag="davs", name="davs")
            sig_t = pool.tile([128, PPT_A], BF16, tag="sig", name="sig")
            fc2_t = pool.tile([128, PPT_A], BF16, tag="fc2", name="fc2")
            ba_t = pool.tile([128, PPT_A], BF16, tag="ba", name="ba")
            f20_t = pool.tile([128, PPT_A], BF16, tag="f20", name="f20")
            fcs_t = pool.tile([128, PPT_R], BF16, tag="fcs", name="fcs")

            # ============ PHASE A: all Sin-table work ============
            for (off_, P_, M_, E, e0) in META_A:
                d01b = pool.tile([128, 2 * E_A], F32, tag="d01", name="d01",
                                 bufs=2)
                base = 128 * 2 * e0
                nc.sync.dma_start(
                    out=d01b[:, 0:2 * E],
                    in_=d01_p[base:base + 128 * 2 * E]
                    .rearrange("(p q) -> p q", p=128))
                s0 = pool.tile([128, E_A], BF16, tag="s0", name="s0", bufs=2)
                s1 = pool.tile([128, E_A], BF16, tag="s1", name="s1", bufs=2)
                act(s0[:, 0:E], d01b[:, 0:E], AF.Sin, scale=SIN_SCALE_A,
                    bias=HALFPI)
                act(s1[:, 0:E], d01b[:, E:2 * E], AF.Sin, scale=SIN_SCALE_A,
                    bias=HALFPI)
                vec.tensor_tensor(prod_t[:, e0:e0 + E], s0[:, 0:E],
                                  s1[:, 0:E], OP.mult)
                vec.tensor_tensor(davs_t[:, e0:e0 + E], d01b[:, 0:E],
                                  d01b[:, E:2 * E], OP.add)
            for (off_, P_, M_, E, e0) in META_R:
                drb = pool.tile([128, E_R], F32, tag="drA", name="drA",
                                bufs=1)
                nc.sync.dma_start(
                    out=drb[:, 0:E],
                    in_=dr_p[128 * e0:128 * (e0 + E)]
                    .rearrange("(p q) -> p q", p=128))
                act(fcs_t[:, e0:e0 + E], drb[:, 0:E], AF.Sin,
                    scale=SIN_SCALE_R, bias=HALFPI)
            s0e = pool.tile([128, EXT_A], BF16, tag="s0e", name="s0e")
            s1e = pool.tile([128, EXT_A], BF16, tag="s1e", name="s1e")
            act(s0e[:, :], ga_all[:, 1, :], AF.Sin, scale=SIN_SCALE_A,
                bias=HALFPI)
            act(s1e[:, :], ga_all[:, 2, :], AF.Sin, scale=SIN_SCALE_A,
                bias=HALFPI)
            prode = pool.tile([128, EXT_A], BF16, tag="prE", name="prE")
            davse = pool.tile([128, EXT_A], BF16, tag="dvE", name="dvE")
            vec.tensor_tensor(prode[:, :], s0e[:, :], s1e[:, :], OP.mult)
            vec.tensor_tensor(davse[:, :], ga_all[:, 1, :], ga_all[:, 2, :],
                              OP.add)
            fcs_all = pool.tile([128, EXT_R], BF16, tag="fcE", name="fcE")
            act(fcs_all[:, :], dr_all[:, :], AF.Sin, scale=SIN_SCALE_R,
                bias=HALFPI)

            # ============ batched B-prep (exp/ln table) ============
            def b_prep(mu_ap, prod_ap, davs_ap, sig_ap, fc2_ap, ba_ap,
                       f20_ap, n, tmptag, nametag):
                tmp1 = pool.tile([128, n], F32, tag=tmptag,
                                 name=nametag + "t1", bufs=2)
                tmp2 = pool.tile([128, n], F32, tag=tmptag,
                                 name=nametag + "t2", bufs=2)
                act(tmp1[:, :], mu_ap, AF.Square)
                act(tmp2[:, :], tmp1[:, :], AF.Ln, scale=-1.0, bias=1.0)
                act(sig_ap, tmp2[:, :], AF.Exp, scale=0.5)
                act(fc2_ap, prod_ap, AF.Square, scale=float(np.sqrt(2.0)))
                act(ba_ap, davs_ap, AF.Exp, scale=B_A_SCALE)
                tmp3 = pool.tile([128, n], F32, tag=tmptag,
                                 name=nametag + "t3", bufs=2)
                act(tmp3[:, :], davs_ap, AF.Square, scale=0.5,
                    bias=-float(SHFA[0]))
                # f2_0 * e^SHIFT_A (the shift is cancelled inside tjf)
                act(f20_ap, tmp3[:, :], AF.Exp, scale=-ETA_A, bias=SHIFT_A)

            HPPT = (PPT_A + 1) // 2
            for ci, (a, b) in enumerate([(0, HPPT), (HPPT, PPT_A)]):
                sl = slice(a, b)
                b_prep(mu_t[:, sl], prod_t[:, sl], davs_t[:, sl],
                       sig_t[:, sl], fc2_t[:, sl], ba_t[:, sl],
                       f20_t[:, sl], b - a, "btmp", f"bp{ci}")
            sige = pool.tile([128, EXT_A], BF16, tag="sgE", name="sgE")
            fc2e = pool.tile([128, EXT_A], BF16, tag="fcE2", name="fcE2")
            bae = pool.tile([128, EXT_A], BF16, tag="baE", name="baE")
            f20e = pool.tile([128, EXT_A], BF16, tag="f20E", name="f20E")
            b_prep(ga_all[:, 0, :], prode[:, :], davse[:, :], sige[:, :],
                   fc2e[:, :], bae[:, :], f20e[:, :], EXT_A, "btmp", "bpe")

            # ============ angular term pipeline (wj-major) ============
            def angular_terms3(mu_ap, sig_ap, fc2_ap, ba_ap, f20_ap, E,
                               terms3, nm):
                """APs are (128, E) views; terms3 is a [128, SUB, E] view."""
                tj = pool.tile([128, NZ, E_A], F32, tag="tj", name="tj" + nm,
                               bufs=1)
                for jj in range(NZ):
                    vec.tensor_scalar(tj[:, jj, 0:E], sig_ap,
                                      0.5 * float(SINZ[jj]), 0.5,
                                      OP.mult, OP.add)
                    vec.scalar_tensor_tensor(tj[:, jj, 0:E], mu_ap,
                                             0.5 * float(COSZ[jj]),
                                             tj[:, jj, 0:E], OP.mult,
                                             OP.add)
                vec.tensor_scalar(tj[:, :, 0:E], tj[:, :, 0:E], 1e-20,
                                  None, OP.max, OP.bypass)
                act(tj[:, :, 0:E], tj[:, :, 0:E], AF.Ln)
                tjf = pool.tile([128, NZ, E_A], BF16, tag="tjf",
                                name="tjf" + nm, bufs=2)
                act(tjf[:, :, 0:E], tj[:, :, 0:E], AF.Exp, scale=ZETA,
                    bias=-SHIFT_A)
                for jj in range(NZ):
                    vec.tensor_tensor(tjf[:, jj, 0:E], tjf[:, jj, 0:E],
                                      fc2_ap, OP.mult)

                f2 = pool.tile([128, NA, E_A], BF16, tag="f2",
                               name="f2" + nm, bufs=1)
                vec.tensor_copy(f2[:, 0, 0:E], f20_ap)
                for w in range(1, NA):
                    vec.scalar_tensor_tensor(f2[:, w, 0:E], ba_ap,
                                             K_A[w - 1], f2[:, w - 1, 0:E],
                                             OP.mult, OP.mult)
                o4 = terms3.rearrange("p (w j) e -> p w j e", w=NA)
                f2v = f2[:, :, 0:E].unsqueeze(2) \
                    .broadcast_to([128, NA, NZ, E])
                tjv = tjf[:, :, 0:E].unsqueeze(1) \
                    .broadcast_to([128, NA, NZ, E])
                vec.tensor_tensor(o4, f2v, tjv, OP.mult)

            def angular_block(bi):
                (off_, P_, M_, E, e0) = META_A[bi]
                terms3 = pool.tile([128, SUB, E_A], BF16, tag="t24",
                                   name="t3", bufs=2)
                sl = slice(e0, e0 + E)
                angular_terms3(mu_t[:, sl], sig_t[:, sl], fc2_t[:, sl],
                               ba_t[:, sl], f20_t[:, sl], E,
                               terms3[:, :, 0:E], f"d{bi}")
                dst = out_a[off_ * SUB:(off_ + P_ * M_) * SUB] \
                    .rearrange("(p q) -> p q", p=P_)
                nc.sync.dma_start(out=dst, in_=terms3[:P_, :, 0:M_])

            # ============ radial term pipeline (log-space chain) ========
            def radial_block(bi):
                (off_, P_, M_, E, e0) = META_R[bi]
                drb = pool.tile([128, E_R], F32, tag="drB", name="drB",
                                bufs=2)
                nc.sync.dma_start(
                    out=drb[:, 0:E],
                    in_=dr_p[128 * e0:128 * (e0 + E)]
                    .rearrange("(p q) -> p q", p=128))
                fin16 = pool.tile([128, M_R * NRBF], BF16, tag="rfin",
                                  name="rfin", bufs=2)
                radial_terms(drb[:, 0:E], fcs_t[:, e0:e0 + E], E, M_,
                             fin16, f"rd{bi}")
                dst = out_r[off_ * NRBF:(off_ + P_ * M_) * NRBF] \
                    .rearrange("(p q) -> p q", p=P_)
                f16v = fin16[:, :].rearrange("p (a b) -> p a b", a=NRBF)
                nc.sync.dma_start(out=dst, in_=f16v[:P_, :, 0:M_])

            def radial_terms(d_ap, fcs_ap, E, M_, fin16, nm):
                """Dense radial: E = 8*M_ entries -> fin16[:, 0:16*M_]
                in r-major layout [p, r, m] (host transposes)."""
                sq = pool.tile([128, E_R], F32, tag="rt0", name="rt0" + nm,
                               bufs=2)
                act(sq[:, 0:E], d_ap, AF.Square, bias=-float(CENTERS_R[0]))
                lnfc = pool.tile([128, E_R], F32, tag="rfc", name="rfc" + nm,
                                 bufs=2)
                act(lnfc[:, 0:E], fcs_ap, AF.Square, scale=0.5)
                act(lnfc[:, 0:E], lnfc[:, 0:E], AF.Ln)
                vv = pool.tile([128, E_R], F32, tag="rbb", name="rbb" + nm,
                               bufs=2)
                vec.tensor_scalar(vv[:, 0:E], d_ap, B_R_SCALE, CV_R,
                                  OP.mult, OP.add)
                logt = pool.tile([128, NRBF, E_R], F32, tag="t24",
                                 name="rlog" + nm, bufs=2)
                vec.scalar_tensor_tensor(logt[:, 0, 0:E], sq[:, 0:E],
                                         -ETA_R, lnfc[:, 0:E], OP.mult,
                                         OP.add)
                for r in range(1, NRBF):
                    eng = vec if r < CHAIN_SPLIT else gps
                    eng.scalar_tensor_tensor(logt[:, r, 0:E], vv[:, 0:E],
                                             KL_R[r - 1],
                                             logt[:, r - 1, 0:E],
                                             OP.add, OP.add)
                terms = pool.tile([128, NRBF, E_R], BF16, tag="rterms",
                                  name="rterms" + nm, bufs=2)
                act(terms[:, :, 0:E], logt[:, :, 0:E], AF.Exp)
                # rank-major entries: fold 6 ranks -> 3 -> 1
                M4 = E // 2
                t4 = pool.tile([128, NRBF, E_R // 2], BF16, tag="rt4",
                               name="rt4" + nm, bufs=1)
                vec.tensor_tensor(t4[:, :, 0:M4], terms[:, :, 0:M4],
                                  terms[:, :, M4:E], OP.add)
                t2t = pool.tile([128, NRBF, E_R // 4], BF16, tag="rt2",
                                name="rt2" + nm, bufs=1)
                tre.tensor_tensor(t2t[:, :, 0:M_], t4[:, :, 0:M_],
                                  t4[:, :, M_:2 * M_], OP.add)
                f16v = fin16[:, :].rearrange("p (a b) -> p a b", a=NRBF)
                tre.tensor_tensor(f16v[:, :, 0:M_], t2t[:, :, 0:M_],
                                  t4[:, :, 2 * M_:3 * M_], OP.add)

            # ---- merged main loop: interleave angular + radial blocks ----
            # (radial first: its recurrence only needs fcs_t, so the DVE has
            # work while the scalar engine finishes b_prep)
            nA, nR = len(META_A), len(META_R)
            merged = []
            fa = fr = 0
            while fa < nA or fr < nR:
                if fr < nR and (fa >= nA or (fr - 2) * nA < fa * nR):
                    merged.append(("R", fr))
                    fr += 1
                else:
                    merged.append(("A", fa))
                    fa += 1
            for kind, bi in merged:
                if kind == "A":
                    angular_block(bi)
                else:
                    radial_block(bi)

            # ============ extras: angular groups ============
            offs_a = layout["offs_a"]
            eoffs_a = []
            eoff = 0
            for (e, n_pad) in layout["table_a"]:
                eoffs_a.append(eoff)
                eoff += (n_pad // 128) * SUB
            for gi, grp in enumerate(layout["groups_a"]):
                g0 = offs_a[grp[0]]
                gE = sum((layout["table_a"][ti][1] // 128)
                         * layout["table_a"][ti][0] for ti in grp)
                terms3 = pool.tile([128, SUB, E_A], BF16, tag="t24",
                                   name=f"t3E{gi}", bufs=2)
                sl = slice(g0, g0 + gE)
                angular_terms3(ga_all[:, 0, sl], sige[:, sl], fc2e[:, sl],
                               bae[:, sl], f20e[:, sl], gE,
                               terms3[:, :, 0:gE], f"e{gi}")
                for ti in grp:
                    e, n_pad = layout["table_a"][ti]
                    rpp = n_pad // 128
                    Ein = rpp * e
                    c0 = offs_a[ti] - g0
                    if e == 1:
                        src = terms3[:, :, c0:c0 + rpp]
                    else:
                        tv = terms3[:, :, c0:c0 + Ein].rearrange(
                            "p s (a b) -> p s a b", b=e)
                        ee = e
                        while ee > 2:
                            tre.tensor_tensor(tv[:, :, :, 0:ee // 2],
                                              tv[:, :, :, 0:ee // 2],
                                              tv[:, :, :, ee // 2:ee],
                                              OP.add)
                            ee //= 2
                        asum = pool.tile([128, SUB, rpp], BF16, tag="asum",
                                         name=f"asum{ti}", bufs=1)
                        tre.tensor_tensor(asum[:, :, :], tv[:, :, :, 0],
                                          tv[:, :, :, 1], OP.add)
                        src = asum[:, :, :]
                    nc.sync.dma_start(
                        out=ext_a[128 * eoffs_a[ti]:
                                  128 * (eoffs_a[ti] + rpp * SUB)]
                        .rearrange("(p q) -> p q", p=128),
                        in_=src)

            # ============ extras: radial groups ============
            offs_r = layout["offs_r"]
            eoffs_r = []
            eoff = 0
            for (e, n_pad) in layout["table_r"]:
                eoffs_r.append(eoff)
                eoff += (n_pad // 128) * NRBF
            for gi, grp in enumerate(layout["groups_r"]):
                g0 = offs_r[grp[0]]
                gE = sum((layout["table_r"][ti][1] // 128)
                         * layout["table_r"][ti][0] for ti in grp)
                sl = slice(g0, g0 + gE)
                sq = pool.tile([128, E_R], F32, tag="rt0", name=f"sqE{gi}",
                               bufs=2)
                act(sq[:, 0:gE], dr_all[:, sl], AF.Square,
                    bias=-float(CENTERS_R[0]))
                lnfc = pool.tile([128, E_R], F32, tag="rfc",
                                 name=f"lfE{gi}", bufs=2)
                act(lnfc[:, 0:gE], fcs_all[:, sl], AF.Square, scale=0.5)
                act(lnfc[:, 0:gE], lnfc[:, 0:gE], AF.Ln)
                vv = pool.tile([128, E_R], F32, tag="rbb", name=f"vvE{gi}",
                               bufs=2)
                vec.tensor_scalar(vv[:, 0:gE], dr_all[:, sl],
                                  B_R_SCALE, CV_R, OP.mult, OP.add)
                logt = pool.tile([128, NRBF, E_R], F32, tag="t24",
                                 name=f"lRE{gi}", bufs=2)
                vec.scalar_tensor_tensor(logt[:, 0, 0:gE], sq[:, 0:gE],
                                         -ETA_R, lnfc[:, 0:gE], OP.mult,
                                         OP.add)
                for r in range(1, NRBF):
                    eng = vec if r < CHAIN_SPLIT else gps
                    eng.scalar_tensor_tensor(logt[:, r, 0:gE], vv[:, 0:gE],
                                             KL_R[r - 1],
                                             logt[:, r - 1, 0:gE],
                                             OP.add, OP.add)
                terms = pool.tile([128, NRBF, E_R], BF16, tag="rterms",
                                  name=f"tRE{gi}", bufs=2)
                act(terms[:, :, 0:gE], logt[:, :, 0:gE], AF.Exp)
                for ti in grp:
                    e, n_pad = layout["table_r"][ti]
                    rpp = n_pad // 128
                    Ein = rpp * e
                    c0 = offs_r[ti] - g0
                    if e == 1:
                        src = terms[:, :, c0:c0 + rpp]
                    else:
                        tv = terms[:, :, c0:c0 + Ein].rearrange(
                            "p c (a b) -> p c a b", b=e)
                        ee = e
                        while ee > 2:
                            tre.tensor_tensor(tv[:, :, :, 0:ee // 2],
                                              tv[:, :, :, 0:ee // 2],
                                              tv[:, :, :, ee // 2:ee],
                                              OP.add)
                            ee //= 2
                        rsum = pool.tile([128, NRBF, rpp], BF16, tag="rsum",
                                         name=f"rsum{ti}", bufs=1)
                        tre.tensor_tensor(rsum[:, :, :], tv[:, :, :, 0],
                                          tv[:, :, :, 1], OP.add)
                        src = rsum[:, :, :]
                    nc.sync.dma_start(
                        out=ext_r[128 * eoffs_r[ti]:
                                  128 * (eoffs_r[ti] + rpp * NRBF)]
                        .rearrange("(p q) -> p q", p=128),
                        in_=src)

    lower_extended_insts(nc)
    _split_excess_waits(nc, 1)
    return nc


def _split_excess_waits(nc, max_waits=1):
    """This neuronxcc build rejects >1 sem-wait per instruction at codegen;
    hoist extras onto preceding event-semaphore carriers."""
    for f in nc.m.functions:
        for b in f.blocks:
            idx = 0
            while idx < len(b.instructions):
                inst = b.instructions[idx]
                si = inst.sync_info
                if si is not None and len(si.on_wait) > max_waits:
                    waits = list(si.on_wait)
                    keep = waits[-max_waits:]
                    head = waits[:-max_waits]
                    at = idx
                    for i0 in range(0, len(head), max_waits):
                        chunk = head[i0:i0 + max_waits]
                        ev = mybir.InstEventSemaphore(
                            name=nc.get_next_instruction_name(), ins=[],
                            outs=[])
                        ev.engine = inst.engine
                        ev.sync_info = mybir.SyncInfo(on_wait=chunk,
                                                      on_update=[])
                        nc.register_instruction(ev)
                        b.instructions.insert(at, ev)
                        at += 1
                        idx += 1
                    si.on_wait = keep
                    inst.sync_info = si
                idx += 1


# --------------------------------------------------------------------------
# Entry point
# --------------------------------------------------------------------------

LAST_RESULT = {}


def kernel(**inputs):
    in_maps, layout, merge = _prepare(inputs)
    nc = build_nc(layout)
    trace = os.environ.get("ANI_TRACE") == "1"
    res = run_bass_kernel_spmd(nc, in_maps, core_ids=list(range(NCORE)),
                               trace=trace)
    LAST_RESULT["exec_time_ns"] = getattr(res, "exec_time_ns", None)
    LAST_RESULT["res"] = res

    parts = []
    for c in range(NCORE):
        rad_raw = np.asarray(res.results[c]["out_r"]).astype(np.float32)
        rad = np.empty((RSLOTS_P, NRBF), np.float32)
        for (off, P_, M_, E, e0) in META_R:
            seg = rad_raw[off * NRBF:(off + P_ * M_) * NRBF] \
                .reshape(P_, NRBF, M_).transpose(0, 2, 1)
            rad[off:off + P_ * M_] = seg.reshape(P_ * M_, NRBF)
        ang_raw = np.asarray(res.results[c]["out_a"]).astype(np.float32)
        ang = np.empty((ASLOTS_P, SUB), np.float32)
        for (off, P_, M_, E, e0) in META_A:
            seg = ang_raw[off * SUB:(off + P_ * M_) * SUB] \
                .reshape(P_, SUB, M_).transpose(0, 2, 1)
            ang[off:off + P_ * M_] = seg.reshape(P_ * M_, SUB)
        er = np.asarray(res.results[c]["ext_r"]).astype(np.float32)
        ea = np.asarray(res.results[c]["ext_a"]).astype(np.float32)
        mrg_r, mrg_a = merge[c]
        eoff = 0
        for ti, (e, n_pad) in enumerate(layout["table_r"]):
            rpp = n_pad // 128
            sums = er[128 * eoff:128 * (eoff + rpp * NRBF)] \
                .reshape(128, NRBF, rpp).transpose(0, 2, 1)
            slots = mrg_r[ti]
            if len(slots):
                q = np.arange(len(slots))
                np.add.at(rad, slots, sums[q % 128, q // 128])
            eoff += rpp * NRBF
        eoff = 0
        for ti, (e, n_pad) in enumerate(layout["table_a"]):
            rpp = n_pad // 128
            sums = ea[128 * eoff:128 * (eoff + rpp * SUB)] \
                .reshape(128, SUB, rpp).transpose(0, 2, 1)
            slots = mrg_a[ti]
            if len(slots):
                q = np.arange(len(slots))
                np.add.at(ang, slots, sums[q % 128, q // 128])
            eoff += rpp * SUB
        parts.append(np.concatenate(
            [rad[:RSLOTS].reshape(NB, S * NRBF),
             ang[:ASLOTS].reshape(NB, NPAIRS * SUB)], axis=1))
    return np.concatenate(parts, axis=0).astype(np.float32)


# revision 37
# speedup vs baseline: 2.5340x; 1.0589x over previous
"""ANI AEV representation kernel for 8 Trainium2 NeuronCores — v3.

Design (data-parallel over atoms, per the sharding hint):
  - Atoms are partitioned into 8 contiguous shards of 6250; each core
    computes its (6250, 1008) AEV slice.
  - Angular: ELL=1 dense layout (one contribution per slot; overflow goes
    through bucketed extra rows summed on device, merged on host).
    Terms are produced in (w,j)-major layout [p, 32, e] so the 8x4 outer
    product is ONE contiguous bf16 tensor_tensor at 2x DVE mode; the host
    permutes (wj, m) -> (m, wj) while unsharding.
  - Radial: ELL=8 dense layout, 48 slots/partition/block (384 entries) so
    the 15-step log-space recurrence amortizes instruction overhead; the
    recurrence tail + reduction tree run on GpSimd to unload the DVE.
  - All Sin evaluations batched in one phase (one ACT table switch);
    exp/ln set stays loaded for the rest.
  - Outputs are bf16 (halves HBM write traffic); host upcasts.
"""

import os
import sys

sys.path.insert(0, "/opt/trn_rl_repo")

import numpy as np

import concourse.bass as bass
import concourse.mybir as mybir
from concourse.library_overlay import lower_extended_insts
from concourse.bass_utils import run_bass_kernel_spmd
from concourse.tile import TileContext

# ---- problem constants (must match reference.py) ----
N = 50000
NCORE = 8
NB = N // NCORE          # 6250 atoms per core
S = 7
NRBF = 16
RC = 0.51
RMIN = 0.08
RCA = 0.35
RAMIN = 0.08
NA = 8
NZ = 4
ETA_R = 1970.0
ETA_A = 1250.0
ZETA = 14.1
NPAIRS = S * (S + 1) // 2   # 28
SUB = NA * NZ               # 32

RSLOTS = NB * S             # 43750 radial slots per core
ASLOTS = NB * NPAIRS        # 175000 angular slots per core

ELL_R = 6
ELL_A = 1
M_R = 64                    # radial slots per partition per block
M_A = 384                   # angular slots per partition per block
E_R = ELL_R * M_R           # 384 entries/partition per radial block
E_A = ELL_A * M_A           # 384 entries/partition per angular block

# gpsimd cannot run scalar_tensor_tensor (walrus backend crash); 16 keeps
# the whole radial recurrence on the DVE.
CHAIN_SPLIT = int(os.environ.get("ANI_CHAIN_SPLIT", "16"))

EBUCKETS = (1, 2, 4, 8, 16)
EMAX = EBUCKETS[-1]

F32 = mybir.dt.float32
BF16 = mybir.dt.bfloat16
AF = mybir.ActivationFunctionType
OP = mybir.AluOpType

CENTERS_R = (RMIN + (RC - RMIN) / NRBF * np.arange(NRBF)).astype(np.float64)
DLT_R = (RC - RMIN) / NRBF
SHFA = (RAMIN + (RCA - RAMIN) / NA * np.arange(NA)).astype(np.float64)
DLT_A = (RCA - RAMIN) / NA
SHFZ = ((np.arange(NZ) + 0.5) * (np.pi / NZ)).astype(np.float64)
COSZ = np.cos(SHFZ)
SINZ = np.sin(SHFZ)

# radial log-space chain: log t_r = log t_{r-1} + v + KL_r with
# v = 2*eta*dlt*(d - c0), KL_r = -eta*dlt^2*(2r-1)
B_R_SCALE = float(2.0 * ETA_R * DLT_R)
CV_R = float(-2.0 * ETA_R * DLT_R * CENTERS_R[0])
KL_R = [float(-ETA_R * DLT_R * DLT_R * (2 * r - 1)) for r in range(1, NRBF)]
# angular f2 multiplicative chain, shifted by e^SHIFT_A to stay normal:
# f2s_w = f2s_{w-1} * Ba * K_w, Ba = exp(eta*dlt*davsum)
K_A = [float(np.exp(-ETA_A * DLT_A * (SHFA[w] + SHFA[w - 1])))
       for w in range(1, NA)]
B_A_SCALE = float(ETA_A * DLT_A)   # applied to davsum = d0 + d1
SHIFT_A = 45.0                     # f2 carries e^+S, tjf carries e^-S

SIN_SCALE_R = float(np.pi / (2.0 * RC))
SIN_SCALE_A = float(np.pi / (2.0 * RCA))
HALFPI = float(np.pi / 2.0)

INERT_D_R = 0.7             # pads: B finite, exp(-eta*(d-c)^2) == 0
INERT_A = np.array([0.0, 0.9, 0.9], np.float32)   # [mu, d0, d1]


def _triu_index_np(num_species):
    s1, s2 = np.triu_indices(num_species)
    ret = np.zeros((num_species, num_species), dtype=np.int64)
    ret[s1, s2] = np.arange(len(s1))
    ret[s2, s1] = np.arange(len(s1))
    return ret


TRIU = _triu_index_np(S)

# --------------------------------------------------------------------------
# Host planning
# --------------------------------------------------------------------------


def _blocks(total, m):
    """Full (128, m) blocks; the tail is a (128, ceil(rem/128)) block over
    padded slot space so every block keeps all 128 partitions busy.
    Returns (blocks, padded_total)."""
    out = []
    off = 0
    n_full = total // (128 * m)
    for _ in range(n_full):
        out.append((off, 128, m))
        off += 128 * m
    rem = total - off
    if rem > 0:
        mt = (rem + 127) // 128
        out.append((off, 128, mt))
        off += 128 * mt
    return out, off


BLOCKS_R, RSLOTS_P = _blocks(RSLOTS, M_R)
BLOCKS_A, ASLOTS_P = _blocks(ASLOTS, M_A)


def _block_meta(blocks, ell):
    meta = []
    e0 = 0
    for (off, P_, M_) in blocks:
        E = ell * M_
        meta.append((off, P_, M_, E, e0))
        e0 += E
    return meta


META_R = _block_meta(BLOCKS_R, ELL_R)
META_A = _block_meta(BLOCKS_A, ELL_A)
PPT_R = sum(m[3] for m in META_R)
PPT_A = sum(m[3] for m in META_A)


def _plan_dense(slots, vals, nslots, ell, blocks, inert_row):
    """First `ell` contributions per slot -> block-aware dense array;
    the rest become extras."""
    order = np.argsort(slots, kind="stable")
    ss = slots[order]
    vs = vals[order]
    counts = np.bincount(ss, minlength=nslots)
    starts = np.zeros(nslots + 1, np.int64)
    np.cumsum(counts, out=starts[1:])
    rank = np.arange(len(ss)) - np.repeat(starts[:-1], counts)

    dense = np.tile(inert_row.astype(np.float32), (nslots * ell, 1))
    keep = rank < ell
    offs = np.array([b[0] for b in blocks])
    bidx = np.searchsorted(offs, ss[keep], side="right") - 1
    boff = offs[bidx]
    bM = np.array([b[2] for b in blocks])[bidx]
    p = (ss[keep] - boff) // bM
    mm = (ss[keep] - boff) % bM
    pos = boff * ell + p * (ell * bM) + rank[keep] * bM + mm
    dense[pos] = vs[keep]
    ex = ~keep
    return dense, (ss[ex], (rank[ex] - ell).astype(np.int64), vs[ex])


def _plan_rows(ex_slot, ex_rank, ex_vals, inert_row):
    """Overflow contributions -> rows of <=EMAX entries bucketed by count.
    Returns {e: (rows (n,e,C) f32, row_slot (n,))}, rows sorted by slot."""
    out = {}
    if len(ex_slot) == 0:
        return out
    row_id = ex_rank // EMAX
    within = ex_rank % EMAX
    key = ex_slot * 64 + row_id
    ukey, uinv = np.unique(key, return_inverse=True)
    u_slot = ukey // 64
    row_n = np.bincount(uinv)
    barr = np.asarray(EBUCKETS)
    row_e = barr[np.searchsorted(barr, row_n)]
    for e in EBUCKETS:
        rows_mask = row_e == e
        nrows = int(rows_mask.sum())
        if nrows == 0:
            continue
        ridx = np.nonzero(rows_mask)[0]
        rmap = np.full(len(ukey), -1, np.int64)
        rmap[ridx] = np.arange(nrows)
        cmask = rmap[uinv] >= 0
        rows = np.tile(inert_row.astype(np.float32), (nrows, e, 1))
        rows[rmap[uinv[cmask]], within[cmask]] = ex_vals[cmask]
        out[e] = (rows, u_slot[ridx])
    return out


def _chunk_table(per_core_rows, chk_entries):
    """Global chunk list [(e, n_pad)] with n_pad rows <= 128*(chk//e),
    plus per-core per-chunk (rows, slots)."""
    table = []
    core_chunks = [[] for _ in per_core_rows]
    for e in EBUCKETS:
        nmax = max((len(rc[e][1]) if e in rc else 0) for rc in per_core_rows)
        if nmax == 0:
            continue
        n_pad_total = ((nmax + 127) // 128) * 128
        cap = 128 * (chk_entries // e)
        start = 0
        while start < n_pad_total:
            n_pad = min(cap, n_pad_total - start)
            table.append((e, n_pad))
            for ci, rc in enumerate(per_core_rows):
                rows, slots = rc.get(e, (np.zeros((0, e, 0), np.float32),
                                         np.zeros(0, np.int64)))
                core_chunks[ci].append((rows[start:start + n_pad],
                                        slots[start:start + n_pad]))
            start += n_pad
    return table, core_chunks


def _sec_device_layout(rows, n_pad, e, inert_row):
    """(n, e, C) rows -> flat (128*C*rpp*e): row q -> partition q%128,
    row-slot q//128; per-partition comp-major."""
    C = len(inert_row)
    n = rows.shape[0]
    full = np.tile(inert_row.astype(np.float32), (n_pad, e, 1))
    if n:
        full[:n] = rows
    rpp = n_pad // 128
    arr = full.reshape(rpp, 128, e, C).transpose(1, 3, 0, 2)
    return np.ascontiguousarray(arr).reshape(-1)


def _prepare(inputs):
    atom_index = np.asarray(inputs["atom_index"])
    pair_indices = np.asarray(inputs["pair_indices"])
    d_ij = np.asarray(inputs["d_ij"])
    r_ij = np.asarray(inputs["r_ij"])
    central = np.asarray(inputs["central_atom_index"])
    p12 = np.asarray(inputs["pair_index12"])
    sign12 = np.asarray(inputs["sign12"])

    i, j = pair_indices[0], pair_indices[1]
    si, sj = atom_index[i], atom_index[j]
    d = d_ij[:, 0].astype(np.float32)

    dest = np.concatenate([i, j])
    osp = np.concatenate([sj, si]).astype(np.int64)
    dval = np.concatenate([d, d]).astype(np.float32)[:, None]

    p0, p1 = p12[0], p12[1]
    v0 = r_ij[p0] * sign12[0].astype(np.float32)[:, None]
    v1 = r_ij[p1] * sign12[1].astype(np.float32)[:, None]
    d0 = d[p0]
    d1 = d[p1]
    mu = 0.95 * np.einsum("ij,ij->i", v0, v1) / (d0 * d1)
    s0 = np.where(sign12[0] == 1, sj[p0], si[p0])
    s1 = np.where(sign12[1] == 1, sj[p1], si[p1])
    cls = TRIU[s0, s1].astype(np.int64)
    geom = np.stack([mu, d0, d1], axis=1).astype(np.float32)

    inert_r = np.array([INERT_D_R], np.float32)
    dense_cores = []
    rows_r_cores = []
    rows_a_cores = []
    for c in range(NCORE):
        base = c * NB
        m = (dest >= base) & (dest < base + NB)
        slot_r = ((dest[m] - base) * S + osp[m]).astype(np.int64)
        dense_r, ex_r = _plan_dense(slot_r, dval[m], RSLOTS_P, ELL_R,
                                    BLOCKS_R, inert_r)
        rows_r_cores.append(_plan_rows(*ex_r, inert_r))

        m = (central >= base) & (central < base + NB)
        slot_a = ((central[m] - base) * NPAIRS + cls[m]).astype(np.int64)
        dense_a, ex_a = _plan_dense(slot_a, geom[m], ASLOTS_P, ELL_A,
                                    BLOCKS_A, INERT_A)
        rows_a_cores.append(_plan_rows(*ex_a, INERT_A))
        dense_cores.append((dense_r, dense_a))

    table_r, chunks_r = _chunk_table(rows_r_cores, E_R)
    table_a, chunks_a = _chunk_table(rows_a_cores, E_A)

    in_maps = []
    merge = []
    for c in range(NCORE):
        dense_r, dense_a = dense_cores[c]
        # mu: global p-major (128, PPT_A); d01: per block [p][comp][E]
        mu_cols = []
        d01_parts = []
        for (off, P_, M_, E, e0) in META_A:
            a = dense_a[off * ELL_A:(off + P_ * M_) * ELL_A].reshape(P_, E, 3)
            mu_cols.append(a[:, :, 0])
            d01_parts.append(np.ascontiguousarray(
                a[:, :, 1:3].transpose(0, 2, 1)).reshape(-1))
        mu_flat = np.ascontiguousarray(
            np.concatenate(mu_cols, axis=1)).reshape(-1)
        d01 = np.concatenate(d01_parts)
        dr_cols = []
        for (off, P_, M_, E, e0) in META_R:
            dr_cols.append(
                dense_r[off * ELL_R:(off + P_ * M_) * ELL_R].reshape(P_, E))
        # global p-major (128, PPT_R) to match the single persistent load
        dr = np.ascontiguousarray(np.concatenate(dr_cols, axis=1)) \
            .reshape(-1)

        gext_parts = []
        mrg_a = []
        for ti, (e, n_pad) in enumerate(table_a):
            rows, rslot = chunks_a[c][ti]
            rpp = n_pad // 128
            gext_parts.append(
                _sec_device_layout(rows, n_pad, e, INERT_A)
                .reshape(128, 3, rpp * e))
            mrg_a.append(rslot)
        drext_parts = []
        mrg_r = []
        for ti, (e, n_pad) in enumerate(table_r):
            rows, rslot = chunks_r[c][ti]
            rpp = n_pad // 128
            drext_parts.append(
                _sec_device_layout(rows, n_pad, e, inert_r)
                .reshape(128, 1, rpp * e))
            mrg_r.append(rslot)

        in_maps.append({
            "mu": mu_flat,
            "d01": d01,
            "dr": dr,
            "gext": (np.ascontiguousarray(
                np.concatenate(gext_parts, axis=2)).reshape(-1)
                if gext_parts else np.zeros(128, np.float32)),
            "drext": (np.ascontiguousarray(
                np.concatenate(drext_parts, axis=2)).reshape(-1)
                if drext_parts else np.zeros(128, np.float32)),
        })
        merge.append((mrg_r, mrg_a))

    def _groups(table, cap):
        """Greedy consecutive chunk groups with sum(rpp*e) <= cap.
        Returns (groups: list[list[chunk idx]], offs: per-chunk EXT offset)."""
        offs = []
        groups = []
        cur = []
        cur_sz = 0
        off = 0
        for ti, (e, n_pad) in enumerate(table):
            sz = (n_pad // 128) * e
            offs.append(off)
            off += sz
            if cur and cur_sz + sz > cap:
                groups.append(cur)
                cur = []
                cur_sz = 0
            cur.append(ti)
            cur_sz += sz
        if cur:
            groups.append(cur)
        return groups, offs

    groups_a, offs_a = _groups(table_a, E_A)
    groups_r, offs_r = _groups(table_r, E_R)

    layout = dict(
        table_r=table_r, table_a=table_a,
        groups_a=groups_a, offs_a=offs_a,
        groups_r=groups_r, offs_r=offs_r,
        ext_a_tot=max(1, sum((n // 128) * e for (e, n) in table_a)),
        ext_r_tot=max(1, sum((n // 128) * e for (e, n) in table_r)),
        ext_r_len=max(1, sum((n // 128) * NRBF for (e, n) in table_r)),
        ext_a_len=max(1, sum((n // 128) * SUB for (e, n) in table_a)),
    )
    return in_maps, layout, merge


# --------------------------------------------------------------------------
# Device kernel builder
# --------------------------------------------------------------------------


def build_nc(layout):
    nc = bass.Bass()
    mu_p = nc.declare_dram_parameter("mu", [128 * PPT_A], F32, isOutput=False)
    d01_p = nc.declare_dram_parameter("d01", [128 * 2 * PPT_A], F32,
                                      isOutput=False)
    dr_p = nc.declare_dram_parameter("dr", [128 * PPT_R], F32, isOutput=False)
    gext_p = nc.declare_dram_parameter("gext", [128 * 3 * layout["ext_a_tot"]],
                                       F32, isOutput=False)
    drext_p = nc.declare_dram_parameter("drext", [128 * layout["ext_r_tot"]],
                                        F32, isOutput=False)
    out_r = nc.declare_dram_parameter("out_r", [RSLOTS_P * NRBF], BF16,
                                      isOutput=True)
    out_a = nc.declare_dram_parameter("out_a", [ASLOTS_P * SUB], BF16,
                                      isOutput=True)
    ext_r = nc.declare_dram_parameter("ext_r", [128 * layout["ext_r_len"]],
                                      BF16, isOutput=True)
    ext_a = nc.declare_dram_parameter("ext_a", [128 * layout["ext_a_len"]],
                                      BF16, isOutput=True)

    bias_vals = [HALFPI, 1.0, -float(CENTERS_R[0]), -float(SHFA[0]),
                 SHIFT_A, -SHIFT_A]
    for k, v in enumerate(sorted(set(bias_vals))):
        t = nc.alloc_sbuf_tensor(f"bconst{k}", [128, 1], F32)
        nc.gpsimd.memset(t.ap(), v)
        nc.const_aps.aps[(F32, v)] = t.ap()
    nc.all_engine_barrier()

    act = nc.scalar.activation
    vec = nc.vector
    gps = nc.gpsimd
    tre = vec if os.environ.get("ANI_TREES", "vec") == "vec" else gps

    with TileContext(nc) as tc:
        with tc.tile_pool(name="main", bufs=1) as pool:
            # ---------------- static loads ----------------
            # DMA order matters: the Sin-phase inputs (dr, d01 blocks) go
            # first so the scalar engine can run all Sin work before the
            # exp/ln table loads; mu/gext are only needed at b_prep.
            EXT_A = layout["ext_a_tot"]
            EXT_R = layout["ext_r_tot"]
            dr_t = pool.tile([128, PPT_R], F32, tag="drt", name="drt")
            nc.sync.dma_start(out=dr_t[:, :],
                              in_=dr_p[:].rearrange("(p q) -> p q", p=128))

            # ---------------- static derived ----------------
            prod_t = pool.tile([128, PPT_A], BF16, tag="prod", name="prod")
            davs_t = pool.tile([128, PPT_A], BF16, tag="davs", name="davs")
            sig_t = pool.tile([128, PPT_A], BF16, tag="sig", name="sig")
            fc2_t = pool.tile([128, PPT_A], BF16, tag="fc2", name="fc2")
            ba_t = pool.tile([128, PPT_A], BF16, tag="ba", name="ba")
            f20_t = pool.tile([128, PPT_A], BF16, tag="f20", name="f20")
            fcs_t = pool.tile([128, PPT_R], BF16, tag="fcs", name="fcs")

            # ============ PHASE A: all Sin-table work ============
            for (off_, P_, M_, E, e0) in META_A:
                d01b = pool.tile([128, 2 * E_A], F32, tag="d01", name="d01",
                                 bufs=2)
                base = 128 * 2 * e0
                nc.sync.dma_start(
                    out=d01b[:, 0:2 * E],
                    in_=d01_p[base:base + 128 * 2 * E]
                    .rearrange("(p q) -> p q", p=128))
                s0 = pool.tile([128, E_A], BF16, tag="s0", name="s0", bufs=2)
                s1 = pool.tile([128, E_A], BF16, tag="s1", name="s1", bufs=2)
                act(s0[:, 0:E], d01b[:, 0:E], AF.Sin, scale=SIN_SCALE_A,
                    bias=HALFPI)
                act(s1[:, 0:E], d01b[:, E:2 * E], AF.Sin, scale=SIN_SCALE_A,
                    bias=HALFPI)
                vec.tensor_tensor(prod_t[:, e0:e0 + E], s0[:, 0:E],
                                  s1[:, 0:E], OP.mult)
                vec.tensor_tensor(davs_t[:, e0:e0 + E], d01b[:, 0:E],
                                  d01b[:, E:2 * E], OP.add)
            for (off_, P_, M_, E, e0) in META_R:
                act(fcs_t[:, e0:e0 + E], dr_t[:, e0:e0 + E], AF.Sin,
                    scale=SIN_SCALE_R, bias=HALFPI)
            ga_all = pool.tile([128, 3, EXT_A], F32, tag="gaall",
                               name="gaall")
            nc.sync.dma_start(
                out=ga_all[:, :, :].rearrange("p a b -> p (a b)"),
                in_=gext_p[:].rearrange("(p q) -> p q", p=128))
            dr_all = pool.tile([128, EXT_R], F32, tag="drall", name="drall")
            nc.sync.dma_start(
                out=dr_all[:, :],
                in_=drext_p[:].rearrange("(p q) -> p q", p=128))
            mu_t = pool.tile([128, PPT_A], F32, tag="mu", name="mu")
            nc.sync.dma_start(out=mu_t[:, :],
                              in_=mu_p[:].rearrange("(p q) -> p q", p=128))
            s0e = pool.tile([128, EXT_A], BF16, tag="s0e", name="s0e")
            s1e = pool.tile([128, EXT_A], BF16, tag="s1e", name="s1e")
            act(s0e[:, :], ga_all[:, 1, :], AF.Sin, scale=SIN_SCALE_A,
                bias=HALFPI)
            act(s1e[:, :], ga_all[:, 2, :], AF.Sin, scale=SIN_SCALE_A,
                bias=HALFPI)
            prode = pool.tile([128, EXT_A], BF16, tag="prE", name="prE")
            davse = pool.tile([128, EXT_A], BF16, tag="dvE", name="dvE")
            vec.tensor_tensor(prode[:, :], s0e[:, :], s1e[:, :], OP.mult)
            vec.tensor_tensor(davse[:, :], ga_all[:, 1, :], ga_all[:, 2, :],
                              OP.add)
            fcs_all = pool.tile([128, EXT_R], BF16, tag="fcE", name="fcE")
            act(fcs_all[:, :], dr_all[:, :], AF.Sin, scale=SIN_SCALE_R,
                bias=HALFPI)

            # ============ batched B-prep (exp/ln table) ============
            def b_prep(mu_ap, prod_ap, davs_ap, sig_ap, fc2_ap, ba_ap,
                       f20_ap, n, tmptag, nametag):
                tmp1 = pool.tile([128, n], F32, tag=tmptag,
                                 name=nametag + "t1", bufs=2)
                tmp2 = pool.tile([128, n], F32, tag=tmptag,
                                 name=nametag + "t2", bufs=2)
                act(tmp1[:, :], mu_ap, AF.Square)
                act(tmp2[:, :], tmp1[:, :], AF.Ln, scale=-1.0, bias=1.0)
                act(sig_ap, tmp2[:, :], AF.Exp, scale=0.5)
                act(fc2_ap, prod_ap, AF.Square, scale=float(np.sqrt(2.0)))
                act(ba_ap, davs_ap, AF.Exp, scale=B_A_SCALE)
                tmp3 = pool.tile([128, n], F32, tag=tmptag,
                                 name=nametag + "t3", bufs=2)
                act(tmp3[:, :], davs_ap, AF.Square, scale=0.5,
                    bias=-float(SHFA[0]))
                # f2_0 * e^SHIFT_A (the shift is cancelled inside tjf)
                act(f20_ap, tmp3[:, :], AF.Exp, scale=-ETA_A, bias=SHIFT_A)

            HPPT = (PPT_A + 1) // 2
            for ci, (a, b) in enumerate([(0, HPPT), (HPPT, PPT_A)]):
                sl = slice(a, b)
                b_prep(mu_t[:, sl], prod_t[:, sl], davs_t[:, sl],
                       sig_t[:, sl], fc2_t[:, sl], ba_t[:, sl],
                       f20_t[:, sl], b - a, "btmp", f"bp{ci}")
            sige = pool.tile([128, EXT_A], BF16, tag="sgE", name="sgE")
            fc2e = pool.tile([128, EXT_A], BF16, tag="fcE2", name="fcE2")
            bae = pool.tile([128, EXT_A], BF16, tag="baE", name="baE")
            f20e = pool.tile([128, EXT_A], BF16, tag="f20E", name="f20E")
            b_prep(ga_all[:, 0, :], prode[:, :], davse[:, :], sige[:, :],
                   fc2e[:, :], bae[:, :], f20e[:, :], EXT_A, "btmp", "bpe")

            # ============ angular term pipeline (wj-major) ============
            def angular_terms3(mu_ap, sig_ap, fc2_ap, ba_ap, f20_ap, E,
                               terms3, nm):
                """APs are (128, E) views; terms3 is a [128, SUB, E] view."""
                tj = pool.tile([128, NZ, E_A], F32, tag="tj", name="tj" + nm,
                               bufs=1)
                for jj in range(NZ):
                    vec.tensor_scalar(tj[:, jj, 0:E], sig_ap,
                                      0.5 * float(SINZ[jj]), 0.5,
                                      OP.mult, OP.add)
                    vec.scalar_tensor_tensor(tj[:, jj, 0:E], mu_ap,
                                             0.5 * float(COSZ[jj]),
                                             tj[:, jj, 0:E], OP.mult,
                                             OP.add)
                vec.tensor_scalar(tj[:, :, 0:E], tj[:, :, 0:E], 1e-20,
                                  None, OP.max, OP.bypass)
                act(tj[:, :, 0:E], tj[:, :, 0:E], AF.Ln)
                tjf = pool.tile([128, NZ, E_A], BF16, tag="tjf",
                                name="tjf" + nm, bufs=1)
                act(tjf[:, :, 0:E], tj[:, :, 0:E], AF.Exp, scale=ZETA,
                    bias=-SHIFT_A)
                for jj in range(NZ):
                    vec.tensor_tensor(tjf[:, jj, 0:E], tjf[:, jj, 0:E],
                                      fc2_ap, OP.mult)

                f2 = pool.tile([128, NA, E_A], BF16, tag="f2",
                               name="f2" + nm, bufs=1)
                vec.tensor_copy(f2[:, 0, 0:E], f20_ap)
                for w in range(1, NA):
                    vec.scalar_tensor_tensor(f2[:, w, 0:E], ba_ap,
                                             K_A[w - 1], f2[:, w - 1, 0:E],
                                             OP.mult, OP.mult)
                o4 = terms3.rearrange("p (w j) e -> p w j e", w=NA)
                f2v = f2[:, :, 0:E].unsqueeze(2) \
                    .broadcast_to([128, NA, NZ, E])
                tjv = tjf[:, :, 0:E].unsqueeze(1) \
                    .broadcast_to([128, NA, NZ, E])
                vec.tensor_tensor(o4, f2v, tjv, OP.mult)

            def angular_block(bi):
                (off_, P_, M_, E, e0) = META_A[bi]
                terms3 = pool.tile([128, SUB, E_A], BF16, tag="t24",
                                   name="t3", bufs=2)
                sl = slice(e0, e0 + E)
                angular_terms3(mu_t[:, sl], sig_t[:, sl], fc2_t[:, sl],
                               ba_t[:, sl], f20_t[:, sl], E,
                               terms3[:, :, 0:E], f"d{bi}")
                dst = out_a[off_ * SUB:(off_ + P_ * M_) * SUB] \
                    .rearrange("(p q) -> p q", p=P_)
                nc.sync.dma_start(out=dst, in_=terms3[:P_, :, 0:M_])

            # ============ radial term pipeline (log-space chain) ========
            def radial_block(bi):
                (off_, P_, M_, E, e0) = META_R[bi]
                fin16 = pool.tile([128, M_R * NRBF], BF16, tag="rfin",
                                  name="rfin", bufs=2)
                radial_terms(dr_t[:, e0:e0 + E], fcs_t[:, e0:e0 + E], E, M_,
                             fin16, f"rd{bi}")
                dst = out_r[off_ * NRBF:(off_ + P_ * M_) * NRBF] \
                    .rearrange("(p q) -> p q", p=P_)
                f16v = fin16[:, :].rearrange("p (a b) -> p a b", a=NRBF)
                nc.sync.dma_start(out=dst, in_=f16v[:P_, :, 0:M_])

            def radial_terms(d_ap, fcs_ap, E, M_, fin16, nm):
                """Dense radial: E = 8*M_ entries -> fin16[:, 0:16*M_]
                in r-major layout [p, r, m] (host transposes)."""
                sq = pool.tile([128, E_R], F32, tag="rt0", name="rt0" + nm,
                               bufs=2)
                act(sq[:, 0:E], d_ap, AF.Square, bias=-float(CENTERS_R[0]))
                lnfc = pool.tile([128, E_R], F32, tag="rfc", name="rfc" + nm,
                                 bufs=2)
                act(lnfc[:, 0:E], fcs_ap, AF.Square, scale=0.5)
                act(lnfc[:, 0:E], lnfc[:, 0:E], AF.Ln)
                vv = pool.tile([128, E_R], F32, tag="rbb", name="rbb" + nm,
                               bufs=2)
                vec.tensor_scalar(vv[:, 0:E], d_ap, B_R_SCALE, CV_R,
                                  OP.mult, OP.add)
                logt = pool.tile([128, NRBF, E_R], F32, tag="t24",
                                 name="rlog" + nm, bufs=2)
                vec.scalar_tensor_tensor(logt[:, 0, 0:E], sq[:, 0:E],
                                         -ETA_R, lnfc[:, 0:E], OP.mult,
                                         OP.add)
                for r in range(1, NRBF):
                    eng = vec if r < CHAIN_SPLIT else gps
                    eng.scalar_tensor_tensor(logt[:, r, 0:E], vv[:, 0:E],
                                             KL_R[r - 1],
                                             logt[:, r - 1, 0:E],
                                             OP.add, OP.add)
                terms = pool.tile([128, NRBF, E_R], BF16, tag="rterms",
                                  name="rterms" + nm, bufs=2)
                act(terms[:, :, 0:E], logt[:, :, 0:E], AF.Exp)
                # rank-major entries: fold 6 ranks -> 3 -> 1
                M4 = E // 2
                t4 = pool.tile([128, NRBF, E_R // 2], BF16, tag="rt4",
                               name="rt4" + nm, bufs=1)
                vec.tensor_tensor(t4[:, :, 0:M4], terms[:, :, 0:M4],
                                  terms[:, :, M4:E], OP.add)
                t2t = pool.tile([128, NRBF, E_R // 4], BF16, tag="rt2",
                                name="rt2" + nm, bufs=1)
                tre.tensor_tensor(t2t[:, :, 0:M_], t4[:, :, 0:M_],
                                  t4[:, :, M_:2 * M_], OP.add)
                f16v = fin16[:, :].rearrange("p (a b) -> p a b", a=NRBF)
                tre.tensor_tensor(f16v[:, :, 0:M_], t2t[:, :, 0:M_],
                                  t4[:, :, 2 * M_:3 * M_], OP.add)

            # ============ extras: angular groups ============
            offs_a = layout["offs_a"]
            eoffs_a = []
            eoff = 0
            for (e, n_pad) in layout["table_a"]:
                eoffs_a.append(eoff)
                eoff += (n_pad // 128) * SUB

            def ext_angular_group(gi):
                grp = layout["groups_a"][gi]
                g0 = offs_a[grp[0]]
                gE = sum((layout["table_a"][ti][1] // 128)
                         * layout["table_a"][ti][0] for ti in grp)
                terms3 = pool.tile([128, SUB, E_A], BF16, tag="t24",
                                   name=f"t3E{gi}", bufs=2)
                sl = slice(g0, g0 + gE)
                angular_terms3(ga_all[:, 0, sl], sige[:, sl], fc2e[:, sl],
                               bae[:, sl], f20e[:, sl], gE,
                               terms3[:, :, 0:gE], f"e{gi}")
                for ti in grp:
                    e, n_pad = layout["table_a"][ti]
                    rpp = n_pad // 128
                    Ein = rpp * e
                    c0 = offs_a[ti] - g0
                    if e == 1:
                        src = terms3[:, :, c0:c0 + rpp]
                    else:
                        tv = terms3[:, :, c0:c0 + Ein].rearrange(
                            "p s (a b) -> p s a b", b=e)
                        ee = e
                        while ee > 2:
                            tre.tensor_tensor(tv[:, :, :, 0:ee // 2],
                                              tv[:, :, :, 0:ee // 2],
                                              tv[:, :, :, ee // 2:ee],
                                              OP.add)
                            ee //= 2
                        asum = pool.tile([128, SUB, rpp], BF16, tag="asum",
                                         name=f"asum{ti}", bufs=1)
                        tre.tensor_tensor(asum[:, :, :], tv[:, :, :, 0],
                                          tv[:, :, :, 1], OP.add)
                        src = asum[:, :, :]
                    nc.sync.dma_start(
                        out=ext_a[128 * eoffs_a[ti]:
                                  128 * (eoffs_a[ti] + rpp * SUB)]
                        .rearrange("(p q) -> p q", p=128),
                        in_=src)

            # ============ extras: radial groups ============
            offs_r = layout["offs_r"]
            eoffs_r = []
            eoff = 0
            for (e, n_pad) in layout["table_r"]:
                eoffs_r.append(eoff)
                eoff += (n_pad // 128) * NRBF

            def ext_radial_group(gi):
                grp = layout["groups_r"][gi]
                g0 = offs_r[grp[0]]
                gE = sum((layout["table_r"][ti][1] // 128)
                         * layout["table_r"][ti][0] for ti in grp)
                sl = slice(g0, g0 + gE)
                sq = pool.tile([128, E_R], F32, tag="rt0", name=f"sqE{gi}",
                               bufs=2)
                act(sq[:, 0:gE], dr_all[:, sl], AF.Square,
                    bias=-float(CENTERS_R[0]))
                lnfc = pool.tile([128, E_R], F32, tag="rfc",
                                 name=f"lfE{gi}", bufs=2)
                act(lnfc[:, 0:gE], fcs_all[:, sl], AF.Square, scale=0.5)
                act(lnfc[:, 0:gE], lnfc[:, 0:gE], AF.Ln)
                vv = pool.tile([128, E_R], F32, tag="rbb", name=f"vvE{gi}",
                               bufs=2)
                vec.tensor_scalar(vv[:, 0:gE], dr_all[:, sl],
                                  B_R_SCALE, CV_R, OP.mult, OP.add)
                logt = pool.tile([128, NRBF, E_R], F32, tag="t24",
                                 name=f"lRE{gi}", bufs=2)
                vec.scalar_tensor_tensor(logt[:, 0, 0:gE], sq[:, 0:gE],
                                         -ETA_R, lnfc[:, 0:gE], OP.mult,
                                         OP.add)
                for r in range(1, NRBF):
                    eng = vec if r < CHAIN_SPLIT else gps
                    eng.scalar_tensor_tensor(logt[:, r, 0:gE], vv[:, 0:gE],
                                             KL_R[r - 1],
                                             logt[:, r - 1, 0:gE],
                                             OP.add, OP.add)
                terms = pool.tile([128, NRBF, E_R], BF16, tag="rterms",
                                  name=f"tRE{gi}", bufs=2)
                act(terms[:, :, 0:gE], logt[:, :, 0:gE], AF.Exp)
                for ti in grp:
                    e, n_pad = layout["table_r"][ti]
                    rpp = n_pad // 128
                    Ein = rpp * e
                    c0 = offs_r[ti] - g0
                    if e == 1:
                        src = terms[:, :, c0:c0 + rpp]
                    else:
                        tv = terms[:, :, c0:c0 + Ein].rearrange(
                            "p c (a b) -> p c a b", b=e)
                        ee = e
                        while ee > 2:
                            tre.tensor_tensor(tv[:, :, :, 0:ee // 2],
                                              tv[:, :, :, 0:ee // 2],
                                              tv[:, :, :, ee // 2:ee],
                                              OP.add)
                            ee //= 2
                        rsum = pool.tile([128, NRBF, rpp], BF16, tag="rsum",
                                         name=f"rsum{ti}", bufs=1)
                        tre.tensor_tensor(rsum[:, :, :], tv[:, :, :, 0],
                                          tv[:, :, :, 1], OP.add)
                        src = rsum[:, :, :]
                    nc.sync.dma_start(
                        out=ext_r[128 * eoffs_r[ti]:
                                  128 * (eoffs_r[ti] + rpp * NRBF)]
                        .rearrange("(p q) -> p q", p=128),
                        in_=src)

            # ---- merged main loop ----
            # Radial first (its recurrence only needs fcs_t, so the DVE has
            # work while the scalar engine finishes b_prep); extras groups
            # spread through the middle so their scalar->vector handoffs
            # overlap dense work instead of serializing at the tail.
            nA, nR = len(META_A), len(META_R)
            nEA = len(layout["groups_a"])
            nER = len(layout["groups_r"])
            dense = []
            fa = fr = 0
            while fa < nA or fr < nR:
                if fr < nR and (fa >= nA or (fr - 2) * nA < fa * nR):
                    dense.append(("R", fr))
                    fr += 1
                else:
                    dense.append(("A", fa))
                    fa += 1
            ext_items = ([("EA", i) for i in range(nEA)]
                         + [("ER", i) for i in range(nER)])
            merged = []
            nd = len(dense)
            for k, it in enumerate(dense):
                merged.append(it)
                # insert extras after ~40% of dense work, evenly
                want = int(len(ext_items) * max(0, k - nd // 3)
                           / max(1, nd - nd // 3))
                while ext_items and len([x for x in merged
                                         if x[0] in ("EA", "ER")]) < want:
                    merged.append(ext_items.pop(0))
            merged.extend(ext_items)
            for kind, bi in merged:
                if kind == "A":
                    angular_block(bi)
                elif kind == "R":
                    radial_block(bi)
                elif kind == "EA":
                    ext_angular_group(bi)
                else:
                    ext_radial_group(bi)

    lower_extended_insts(nc)
    _split_excess_waits(nc, 1)
    return nc


def _split_excess_waits(nc, max_waits=1):
    """This neuronxcc build rejects >1 sem-wait per instruction at codegen;
    hoist extras onto preceding event-semaphore carriers."""
    for f in nc.m.functions:
        for b in f.blocks:
            idx = 0
            while idx < len(b.instructions):
                inst = b.instructions[idx]
                si = inst.sync_info
                if si is not None and len(si.on_wait) > max_waits:
                    waits = list(si.on_wait)
                    keep = waits[-max_waits:]
                    head = waits[:-max_waits]
                    at = idx
                    for i0 in range(0, len(head), max_waits):
                        chunk = head[i0:i0 + max_waits]
                        ev = mybir.InstEventSemaphore(
                            name=nc.get_next_instruction_name(), ins=[],
                            outs=[])
                        ev.engine = inst.engine
                        ev.sync_info = mybir.SyncInfo(on_wait=chunk,
                                                      on_update=[])
                        nc.register_instruction(ev)
                        b.instructions.insert(at, ev)
                        at += 1
                        idx += 1
                    si.on_wait = keep
                    inst.sync_info = si
                idx += 1


# --------------------------------------------------------------------------
# Entry point
# --------------------------------------------------------------------------

LAST_RESULT = {}


def kernel(**inputs):
    in_maps, layout, merge = _prepare(inputs)
    nc = build_nc(layout)
    trace = os.environ.get("ANI_TRACE") == "1"
    res = run_bass_kernel_spmd(nc, in_maps, core_ids=list(range(NCORE)),
                               trace=trace)
    LAST_RESULT["exec_time_ns"] = getattr(res, "exec_time_ns", None)
    LAST_RESULT["res"] = res

    parts = []
    for c in range(NCORE):
        rad_raw = np.asarray(res.results[c]["out_r"]).astype(np.float32)
        rad = np.empty((RSLOTS_P, NRBF), np.float32)
        for (off, P_, M_, E, e0) in META_R:
            seg = rad_raw[off * NRBF:(off + P_ * M_) * NRBF] \
                .reshape(P_, NRBF, M_).transpose(0, 2, 1)
            rad[off:off + P_ * M_] = seg.reshape(P_ * M_, NRBF)
        ang_raw = np.asarray(res.results[c]["out_a"]).astype(np.float32)
        ang = np.empty((ASLOTS_P, SUB), np.float32)
        for (off, P_, M_, E, e0) in META_A:
            seg = ang_raw[off * SUB:(off + P_ * M_) * SUB] \
                .reshape(P_, SUB, M_).transpose(0, 2, 1)
            ang[off:off + P_ * M_] = seg.reshape(P_ * M_, SUB)
        er = np.asarray(res.results[c]["ext_r"]).astype(np.float32)
        ea = np.asarray(res.results[c]["ext_a"]).astype(np.float32)
        mrg_r, mrg_a = merge[c]
        eoff = 0
        for ti, (e, n_pad) in enumerate(layout["table_r"]):
            rpp = n_pad // 128
            sums = er[128 * eoff:128 * (eoff + rpp * NRBF)] \
                .reshape(128, NRBF, rpp).transpose(0, 2, 1)
            slots = mrg_r[ti]
            if len(slots):
                q = np.arange(len(slots))
                np.add.at(rad, slots, sums[q % 128, q // 128])
            eoff += rpp * NRBF
        eoff = 0
        for ti, (e, n_pad) in enumerate(layout["table_a"]):
            rpp = n_pad // 128
            sums = ea[128 * eoff:128 * (eoff + rpp * SUB)] \
                .reshape(128, SUB, rpp).transpose(0, 2, 1)
            slots = mrg_a[ti]
            if len(slots):
                q = np.arange(len(slots))
                np.add.at(ang, slots, sums[q % 128, q // 128])
            eoff += rpp * SUB
        parts.append(np.concatenate(
            [rad[:RSLOTS].reshape(NB, S * NRBF),
             ang[:ASLOTS].reshape(NB, NPAIRS * SUB)], axis=1))
    return np.concatenate(parts, axis=0).astype(np.float32)
